# revision 1
# baseline (speedup 1.0000x reference)
"""Trainium2 Bass kernel for nn_EncoderBlock (B=4, S=1024, D=1024, H=16, DFF=4096).

Sharding: 8 cores = 4 batches x 2 sequence-halves; each core produces the
block output for its 512 "own" tokens. Attention needs K/V for the batch's
full sequence, so the K/V-stream projections run on all 1024 tokens on both
cores of a batch pair (duplicated) -- zero inter-core communication.

Layouts: activations feature-major ([feature, token], features on SBUF
partitions) so weights are stationary matmul operands in natural [in, out]
layout. Matmuls in bf16 (f32 PSUM accumulation); weights are cast to bf16 and
prepacked on the host into SBUF tile layouts so every weight DMA is one large
contiguous transfer; x is additionally passed as bf16 so the feature-major
transposes go through the DMA transpose engine instead of the PE.

Attention per head: scores key-major (s[k_tok, q_tok]); softmax is
unnormalized exp (scores ~N(0, 0.03) here, no max subtraction needed);
denominators come from an appended ones-column on the V stationary operand;
normalization multiplies the head output by a PE-broadcast reciprocal (the
tiny broadcast/bias matmuls run in float32r). The per-head-pair K/Q
projections are interleaved with the attention loop so the ACT-bound exp
stream hides under projection matmuls.

SBUF pool lifetimes are LIFO per side: transients nest on the left;
attention-persistent tensors stack on the right. One global PSUM pool with
tag rotation (ps:4 + ops:2 + bc:2 = 8 banks).
"""

import math
import numpy as np

B, S, D, H = 4, 1024, 1024, 16
HD = D // H
DFF = 4 * D
T = S // 2
P = 128
NT = T // P     # 4
NS = S // P     # 8
ND = D // P     # 8
NHP = H // 2    # 8
NF = DFF // P   # 32
EPS = 1e-5
SCL = 1.0 / math.sqrt(D)

_CACHE = {}


def _build():
    import concourse.mybir as mybir
    import concourse.tile as tile
    from concourse import bacc
    from concourse.masks import make_identity
    from contextlib import ExitStack

    F32 = mybir.dt.float32
    F32R = mybir.dt.float32r
    BF16 = mybir.dt.bfloat16
    AF = mybir.ActivationFunctionType
    OP = mybir.AluOpType

    nc = bacc.Bacc(None, target_bir_lowering=False, debug=False)

    with tile.TileContext(nc) as tc:
        es = ExitStack()
        dram = es.enter_context(tc.tile_pool(name="dram", bufs=1, space="DRAM"))

        def din(name, shape, dt=BF16):
            return dram.tile(shape, dt, kind="ExternalInput", name=name, uniquify=False)

        x_bf = din("x_bf", [S, D])            # batch's full sequence, bf16
        xo_bf = din("xo_bf", [T, D])          # own tokens, bf16
        x_own = din("x_own", [T, D], F32)     # own tokens, f32 (residual)
        Wk = din("Wk", [D, D]); Wq = din("Wq", [D, D]); Wv = din("Wv", [D, D])
        Whq = din("Whq_p", [NHP, P, D])       # [hp, p, (c h' e)] prepacked
        Whk = din("Whk_p", [NHP, P, D])
        Whv = din("Whv_p", [ND, P, D])        # [c, p, (h e)] prepacked
        Wo = din("Wo", [D, D])
        W1 = din("W1_p", [8, D, 512])         # [blk, d, j] prepacked
        W2 = din("W2", [DFF, D])
        bk = din("bk", [D], F32); bq = din("bq", [D], F32); bv = din("bv", [D], F32)
        bhq = din("bhq", [H, HD], F32); bhk = din("bhk", [H, HD], F32)
        bhv = din("bhv", [H, HD], F32R)
        bo = din("bo", [D], F32R); b1 = din("b1", [DFF], F32); b2 = din("b2", [D], F32R)
        out = dram.tile([T, D], F32, kind="ExternalOutput", name="out", uniquify=False)

        # ---------------- constants / psum ----------------
        const = es.enter_context(tc.tile_pool(name="const", bufs=1))
        ident = const.tile([P, P], F32, name="ident")
        make_identity(nc, ident)
        ones_f32 = const.tile([P, 16], F32, name="ones_f32")
        nc.vector.memset(ones_f32[:], 1.0)
        onesf2 = const.tile([P, P], F32, name="onesf2")
        nc.vector.memset(onesf2[:], 1.0)
        ones_r = const.tile([P, P], F32R, name="ones_r")
        nc.scalar.copy(ones_r[:], onesf2[:])
        eps_t = const.tile([P, 1], F32, name="eps_t")
        nc.vector.memset(eps_t[:], EPS)

        bo_rt = const.tile([1, D], F32R, name="bo_rt")
        nc.gpsimd.dma_start(out=bo_rt[:], in_=bo[:].rearrange("(o d) -> o d", o=1))
        b2_rt = const.tile([1, D], F32R, name="b2_rt")
        nc.gpsimd.dma_start(out=b2_rt[:], in_=b2[:].rearrange("(o d) -> o d", o=1))
        bhv_rt = const.tile([1, D], F32R, name="bhv_rt")
        nc.gpsimd.dma_start(out=bhv_rt[:], in_=bhv[:].rearrange("(o h) e -> o (h e)", o=1))
        bo_r, b2_r, bhv_r = bo_rt[:], b2_rt[:], bhv_rt[:]

        def bias_cols(name, vec, ncols):
            t = const.tile([P, ncols], F32, name=name)
            nc.gpsimd.dma_start(out=t[:], in_=vec.rearrange("(m p) -> p m", p=P))
            return t

        bk_t = bias_cols("bk_t", bk[:], ND)
        bq_t = bias_cols("bq_t", bq[:], ND)
        bv_t = bias_cols("bv_t", bv[:], ND)
        bhq_t = bias_cols("bhq_t", bhq[:].rearrange("h e -> (h e)"), NHP)
        bhk_t = bias_cols("bhk_t", bhk[:].rearrange("h e -> (h e)"), NHP)
        b1_t = bias_cols("b1_t", b1[:], NF)

        ln_p = es.enter_context(tc.tile_pool(name="ln_p", bufs=3))
        psum = es.enter_context(tc.tile_pool(name="psum", bufs=1, space="PSUM"))

        def ps_tile(name, shape=(P, 512), tag="ps", bufs=4):
            return psum.tile(list(shape), F32, name=name, tag=tag, bufs=bufs)

        dma_i = [0]

        def dma(out_, in_):
            """Strict round-robin across the three DMA issue paths."""
            eng = (nc.scalar, nc.gpsimd, nc.sync)[dma_i[0] % 3]
            dma_i[0] += 1
            eng.dma_start(out=out_, in_=in_)

        ev_i = [0]
        ev_dve_only = [False]

        def evict(dst, src, bias=None):
            """PSUM -> SBUF eviction: 2 of 3 on DVE, 1 of 3 on ACT."""
            i = ev_i[0]; ev_i[0] += 1
            if i % 3 == 2 and not ev_dve_only[0]:
                if bias is None:
                    nc.scalar.copy(dst, src)
                else:
                    nc.scalar.activation(dst, src, AF.Identity, bias=bias)
            else:
                if bias is None:
                    nc.vector.tensor_copy(dst, src)
                else:
                    nc.vector.tensor_scalar_add(dst, src, bias)

        # right-side persistent pools (bottom: longest-lived)
        posb = ExitStack()
        osb_pool = posb.enter_context(tc.tile_pool(name="osb_pool", bufs=1, side="right"))
        o_sb = [osb_pool.tile([P, T], BF16, name=f"o_sb{hp}") for hp in range(NHP)]
        pva = ExitStack()
        va_pool = pva.enter_context(tc.tile_pool(name="va_pool", bufs=1, side="right"))
        v_aug = [va_pool.tile([P, H * (HD + 1)], BF16, name=f"vaug{i}") for i in range(NS)]
        pkt = ExitStack()
        kt_pool = pkt.enter_context(tc.tile_pool(name="kt_pool", bufs=1, side="right"))
        k_t = [kt_pool.tile([P, S], BF16, name=f"kh_o{m}") for m in range(NHP)]
        pqt = ExitStack()
        qt_pool = pqt.enter_context(tc.tile_pool(name="qt_pool", bufs=1, side="right"))
        q_t = [qt_pool.tile([P, T], BF16, name=f"qh_o{m}") for m in range(NHP)]

        # left-side long-lived: qo/ko (read inside the attention loop)
        p_qo = ExitStack()
        qo_pool = p_qo.enter_context(tc.tile_pool(name="qo_pool", bufs=1))
        p_ko = ExitStack()
        ko_pool = p_ko.enter_context(tc.tile_pool(name="ko_pool", bufs=1))

        # ================= Phase A: transpose x via DMA xbar =================
        pxf = ExitStack()
        xf_p = pxf.enter_context(tc.tile_pool(name="xf_p", bufs=1))
        xf_t = [xf_p.tile([P, S], BF16, name=f"xf_t{j}") for j in range(ND)]
        pxo = ExitStack()
        xo_p = pxo.enter_context(tc.tile_pool(name="xo_p", bufs=1))
        xo_t = [xo_p.tile([P, T], BF16, name=f"xo_t{j}") for j in range(ND)]
        for j in range(ND):
            nc.sync.dma_start(out=xf_t[j][:], in_=x_bf[:, j * P:(j + 1) * P],
                              transpose=True)
            nc.scalar.dma_start(out=xo_t[j][:], in_=xo_bf[:, j * P:(j + 1) * P],
                                transpose=True)

        # =============== dense projection helper ===============
        def wproj(name, w_dram, src_tiles, n_tok, bias_col, pool_out, es_phase):
            """Dense [D, D] projection, feature-major output (BF16)."""
            wp = es_phase.enter_context(tc.tile_pool(name=f"w_{name}", bufs=1))
            outs = [pool_out.tile([P, n_tok], BF16, name=f"{name}_o{m}") for m in range(ND)]
            w_sb = []
            for k in range(ND):
                wt = wp.tile([P, D], BF16, name=f"w_{name}{k}")
                dma(wt[:], w_dram[k * P:(k + 1) * P, :])
                w_sb.append(wt)
            for m in range(ND):
                for n in range(n_tok // 512):
                    ps = ps_tile(f"ps_{name}{m}_{n}")
                    for k in range(ND):
                        nc.tensor.matmul(ps[:], w_sb[k][:, m * P:(m + 1) * P],
                                         src_tiles[k][:, n * 512:(n + 1) * 512],
                                         start=(k == 0), stop=(k == ND - 1))
                    evict(outs[m][:, n * 512:(n + 1) * 512], ps[:],
                          bias=bias_col[:, m:m + 1])
            return outs

        # =============== Phase B0: Q-stream outer (only needs xo_t) ===============
        b5s = ExitStack()
        ko_t = wproj("ko", Wk, xo_t, T, bk_t, ko_pool, b5s)
        b5s.close()
        pxo.close()

        # =============== Phase B1: V stream -> v_aug ===============
        p_vo = ExitStack()
        vo_pool = p_vo.enter_context(tc.tile_pool(name="vo_pool", bufs=1))
        b1s = ExitStack()
        vo_t = wproj("vo", Wv, xf_t, S, bv_t, vo_pool, b1s)
        b1s.close()

        b2s = ExitStack()
        whv_p = b2s.enter_context(tc.tile_pool(name="whv", bufs=1))
        whv_sb = []
        for k in range(ND):
            wt = whv_p.tile([P, D], BF16, name=f"whv{k}")
            dma(wt[:], Whv[k])
            whv_sb.append(wt)
        for i in range(NS):
            for n in range(2):
                ps = ps_tile(f"vkm{i}_{n}")
                for k in range(ND):
                    nc.tensor.matmul(ps[:], vo_t[k][:, i * P:(i + 1) * P],
                                     whv_sb[k][:, n * 512:(n + 1) * 512],
                                     start=(k == 0), stop=False)
                nc.tensor.matmul(ps[:], ones_r[:1, 0:P], bhv_r[:, n * 512:(n + 1) * 512],
                                 start=False, stop=True)
                dst = v_aug[i][:].rearrange("p (h e) -> p h e", e=HD + 1)
                evict(dst[:, 8 * n:8 * (n + 1), 0:HD],
                      ps[:].rearrange("p (h e) -> p h e", e=HD))
            dst = v_aug[i][:].rearrange("p (h e) -> p h e", e=HD + 1)
            nc.vector.tensor_copy(dst[:, :, HD:HD + 1],
                                  ones_f32[:, 0:H].rearrange("p (h o) -> p h o", o=1))
        b2s.close()
        p_vo.close()

        # =============== Phase B2/B3: outer projections ===============
        b3s = ExitStack()
        qo_t = wproj("qo", Wq, xf_t, S, bq_t, qo_pool, b3s)
        b3s.close()
        pxf.close()

        # ====== interleaved loop: per head pair, K/Q head proj + attention ======
        pc = ExitStack()
        whk_p = pc.enter_context(tc.tile_pool(name="whk_p", bufs=NHP))
        whq_p = pc.enter_context(tc.tile_pool(name="whq_p", bufs=NHP))
        pkm_p = pc.enter_context(tc.tile_pool(name="pkm", bufs=32))
        den_p = pc.enter_context(tc.tile_pool(name="den_p", bufs=3))
        ev_dve_only[0] = True
        for hp in range(NHP):
            # k_t[hp]: per-head K projection over the full sequence
            wtk = whk_p.tile([P, D], BF16, name=f"whk{hp}", tag="whk")
            dma(wtk[:], Whk[hp])
            for n in range(2):
                ps = ps_tile(f"ps_kh{hp}_{n}")
                for k in range(ND):
                    nc.tensor.matmul(ps[:], wtk[:, k * P:(k + 1) * P],
                                     qo_t[k][:, n * 512:(n + 1) * 512],
                                     start=(k == 0), stop=(k == ND - 1))
                evict(k_t[hp][:, n * 512:(n + 1) * 512], ps[:],
                      bias=bhk_t[:, hp:hp + 1])
            # q_t[hp]: per-head Q projection over own tokens
            wtq = whq_p.tile([P, D], BF16, name=f"whq{hp}", tag="whq")
            dma(wtq[:], Whq[hp])
            ps = ps_tile(f"ps_qh{hp}")
            for k in range(ND):
                nc.tensor.matmul(ps[:], wtq[:, k * P:(k + 1) * P], ko_t[k][:],
                                 start=(k == 0), stop=(k == ND - 1))
            evict(q_t[hp][:], ps[:], bias=bhq_t[:, hp:hp + 1])

            # attention for the two heads of this pair
            for h in (2 * hp, 2 * hp + 1):
                hl = (h % 2) * HD
                p_km = []
                for i in range(NS):
                    ps = ps_tile(f"sc{h}_{i}")
                    nc.tensor.matmul(ps[:], k_t[hp][hl:hl + HD, i * P:(i + 1) * P],
                                     q_t[hp][hl:hl + HD, :], start=True, stop=True)
                    pk = pkm_p.tile([P, T], BF16, name=f"pkm{h}_{i}", tag="pkm")
                    nc.scalar.activation(pk[:], ps[:], AF.Exp, scale=SCL)
                    p_km.append(pk)
                ops = ps_tile(f"ops{h}", shape=(HD + 1, T), tag="ops", bufs=2)
                for i in range(NS):
                    nc.tensor.matmul(ops[:], v_aug[i][:, h * (HD + 1):(h + 1) * (HD + 1)],
                                     p_km[i][:], start=(i == 0), stop=(i == NS - 1))
                den = den_p.tile([1, T], F32R, name=f"den{h}", tag="den")
                with nc.allow_low_precision(reason="f32r is 4-byte f32 storage"):
                    nc.vector.reciprocal(den[:], ops[HD:HD + 1, :])
                bc = ps_tile(f"bc{h}", shape=(HD, T), tag="bc", bufs=2)
                nc.tensor.matmul(bc[:], ones_r[:1, 0:HD], den[:], start=True, stop=True)
                bcs = den_p.tile([HD, T], F32, name=f"bcs{h}", tag="bcs")
                nc.vector.tensor_copy(bcs[:], bc[:])
                nc.vector.tensor_tensor(o_sb[hp][hl:hl + HD, :], ops[0:HD, :], bcs[:],
                                        op=OP.mult)
        ev_dve_only[0] = False
        pc.close()
        pqt.close(); pkt.close(); pva.close()
        p_ko.close(); p_qo.close()

        # =============== Phase D: output proj + residual + LN1 ===============
        pr1 = ExitStack()
        r1_pool = pr1.enter_context(tc.tile_pool(name="r1_pool", bufs=1))
        r1 = [r1_pool.tile([P, D], F32, name=f"r1_{i}") for i in range(NT)]
        r1_t = [r1_pool.tile([P, T], BF16, name=f"r1t{j}") for j in range(ND)]

        pd = ExitStack()
        wo_p = pd.enter_context(tc.tile_pool(name="wo", bufs=1))
        wo_sb = []
        for k in range(ND):
            wt = wo_p.tile([P, D], BF16, name=f"wo{k}")
            dma(wt[:], Wo[k * P:(k + 1) * P, :])
            wo_sb.append(wt)
        x_tok = [wo_p.tile([P, D], F32, name=f"x_tok{i}") for i in range(NT)]
        for i in range(NT):
            dma(x_tok[i][:], x_own[i * P:(i + 1) * P, :])

        def layernorm(tag, i, pre, dst):
            """dst = LN(pre) along free dim (D=1024). pre: [P, D] f32 SBUF."""
            st = ln_p.tile([P, 12], F32, name=f"st{tag}{i}", tag="st")
            nc.vector.bn_stats(st[:, 0:6], pre[:, 0:512])
            nc.vector.bn_stats(st[:, 6:12], pre[:, 512:1024])
            ag = ln_p.tile([P, 2], F32, name=f"ag{tag}{i}", tag="ag")
            nc.vector.bn_aggr(ag[:], st[:].rearrange("p (n s) -> p n s", n=2))
            sd = ln_p.tile([P, 1], F32, name=f"sd{tag}{i}", tag="sd")
            nc.scalar.activation(sd[:], ag[:, 1:2], AF.Sqrt, bias=eps_t[:])
            rs = ln_p.tile([P, 1], F32, name=f"rs{tag}{i}", tag="rs")
            nc.vector.reciprocal(rs[:], sd[:])
            nc.vector.tensor_scalar(dst, pre[:], ag[:, 0:1], rs[:],
                                    op0=OP.subtract, op1=OP.mult)

        for i in range(NT):
            pre = wo_p.tile([P, D], F32, name=f"pre1_{i}", tag="pre1", bufs=2)
            for n in range(2):
                ps = ps_tile(f"at{i}_{n}")
                for k in range(ND):
                    nc.tensor.matmul(ps[:], o_sb[k][:, i * P:(i + 1) * P],
                                     wo_sb[k][:, n * 512:(n + 1) * 512],
                                     start=(k == 0), stop=False)
                nc.tensor.matmul(ps[:], ones_r[:1, 0:P], bo_r[:, n * 512:(n + 1) * 512],
                                 start=False, stop=True)
                nc.vector.tensor_tensor(pre[:, n * 512:(n + 1) * 512], ps[:],
                                        x_tok[i][:, n * 512:(n + 1) * 512], op=OP.add)
            layernorm("r", i, pre, r1[i][:])

        for j in range(ND):
            for i in range(NT):
                tp = ps_tile(f"r1tp{j}_{i}", shape=(P, P), tag="ops", bufs=2)
                nc.tensor.transpose(tp[:P, :P], r1[i][:, j * P:(j + 1) * P], ident[:])
                evict(r1_t[j][:, i * P:(i + 1) * P], tp[:P, :P])
        pd.close()
        posb.close()

        # =============== Phase E: FFN ===============
        pe1 = ExitStack()
        ht_pool = pe1.enter_context(tc.tile_pool(name="ht_pool", bufs=1))
        h_t = [ht_pool.tile([P, T], BF16, name=f"h_t{m}") for m in range(NF)]
        e1s = ExitStack()
        w1_p = e1s.enter_context(tc.tile_pool(name="w1_p", bufs=24))
        for blk in range(8):            # dff blocks of 512
            w1_sb = []
            for k in range(ND):
                wt = w1_p.tile([P, 512], BF16, name=f"w1_{blk}_{k}", tag="w1")
                dma(wt[:], W1[blk, k * P:(k + 1) * P, :])
                w1_sb.append(wt)
            for mm in range(4):         # 128-chunks within the block
                m = blk * 4 + mm
                ps = ps_tile(f"ff1_{m}")
                for k in range(ND):
                    nc.tensor.matmul(ps[:], w1_sb[k][:, mm * P:(mm + 1) * P],
                                     r1_t[k][:], start=(k == 0), stop=(k == ND - 1))
                nc.scalar.activation(h_t[m][:], ps[:], AF.Gelu, bias=b1_t[:, m:m + 1])
        e1s.close()

        e2s = ExitStack()
        w2_p = e2s.enter_context(tc.tile_pool(name="w2_p", bufs=12))
        out_p = e2s.enter_context(tc.tile_pool(name="out_p", bufs=3))
        ff_ps = []
        for i in range(NT):
            for n in range(2):
                tag, bufs = [("ps", 4), ("ps", 4), ("ps", 4), ("ps", 4),
                             ("ops", 2), ("ops", 2), ("bc", 2), ("bc", 2)][i * 2 + n]
                ff_ps.append(ps_tile(f"ff2_{i}_{n}", shape=(P, 512), tag=tag, bufs=bufs))
        for k in range(NF):
            wt = w2_p.tile([P, D], BF16, name=f"w2_{k}", tag="w2")
            dma(wt[:], W2[k * P:(k + 1) * P, :])
            for i in range(NT):
                for n in range(2):
                    nc.tensor.matmul(ff_ps[i * 2 + n][:], h_t[k][:, i * P:(i + 1) * P],
                                     wt[:, n * 512:(n + 1) * 512],
                                     start=(k == 0), stop=False)
        for i in range(NT):
            pre = out_p.tile([P, D], F32, name=f"pre2_{i}", tag="pre2")
            for n in range(2):
                nc.tensor.matmul(ff_ps[i * 2 + n][:], ones_r[:1, 0:P],
                                 b2_r[:, n * 512:(n + 1) * 512], start=False, stop=True)
                nc.vector.tensor_tensor(pre[:, n * 512:(n + 1) * 512], ff_ps[i * 2 + n][:],
                                        r1[i][:, n * 512:(n + 1) * 512],
                                        op=OP.add)
            o_sb2 = out_p.tile([P, D], F32, name=f"osb2_{i}", tag="osb2")
            layernorm("o", i, pre, o_sb2[:])
            nc.sync.dma_start(out=out[i * P:(i + 1) * P, :], in_=o_sb2[:])
        e2s.close()
        pe1.close()
        pr1.close()

        es.close()
    nc.compile()
    return nc


def _get_program():
    if "nc" not in _CACHE:
        _CACHE["nc"] = _build()
    return _CACHE["nc"]


def _prepack(inputs):
    """Cast weights to bf16 and prepack into SBUF tile layouts."""
    import ml_dtypes
    bf16 = ml_dtypes.bfloat16

    def b(a):
        return np.ascontiguousarray(np.asarray(a, dtype=np.float32).astype(bf16))

    Whq = np.asarray(inputs["Whq"], dtype=np.float32)
    Whk = np.asarray(inputs["Whk"], dtype=np.float32)
    Whv = np.asarray(inputs["Whv"], dtype=np.float32)
    W1 = np.asarray(inputs["W1"], dtype=np.float32)
    # [hp, p, (c h' e)]: Whq_p[hp, p, c*128+h'*64+e] = Whq[2hp+h', c*128+p, e]
    whq_p = b(Whq.reshape(NHP, 2, ND, P, HD).transpose(0, 3, 2, 1, 4).reshape(NHP, P, D))
    whk_p = b(Whk.reshape(NHP, 2, ND, P, HD).transpose(0, 3, 2, 1, 4).reshape(NHP, P, D))
    # [c, p, (h e)]: Whv_p[c, p, h*64+e] = Whv[h, c*128+p, e]
    whv_p = b(Whv.reshape(H, ND, P, HD).transpose(1, 2, 0, 3).reshape(ND, P, D))
    # [blk, d, j]
    w1_p = b(W1.reshape(D, 8, 512).transpose(1, 0, 2))
    f32 = lambda n: np.ascontiguousarray(inputs[n], dtype=np.float32)
    return {
        "Wk": b(inputs["Wk"]), "Wq": b(inputs["Wq"]), "Wv": b(inputs["Wv"]),
        "Wo": b(inputs["Wo"]), "W2": b(inputs["W2"]),
        "Whq_p": whq_p, "Whk_p": whk_p, "Whv_p": whv_p, "W1_p": w1_p,
        "bk": f32("bk"), "bq": f32("bq"), "bv": f32("bv"),
        "bhq": f32("bhq"), "bhk": f32("bhk"), "bhv": f32("bhv"),
        "bo": f32("bo"), "b1": f32("b1"), "b2": f32("b2"),
    }


def _in_maps(inputs):
    import ml_dtypes
    x = np.ascontiguousarray(inputs["x"], dtype=np.float32)
    x_bf = x.astype(ml_dtypes.bfloat16)
    wmap = _prepack(inputs)
    in_maps = []
    for c in range(8):
        b_, half = c // 2, c % 2
        m = dict(wmap)
        m["x_bf"] = x_bf[b_]
        m["xo_bf"] = np.ascontiguousarray(x_bf[b_, half * T:(half + 1) * T])
        m["x_own"] = x[b_, half * T:(half + 1) * T]
        in_maps.append(m)
    return in_maps


def kernel(**inputs):
    from concourse.bass_utils import run_bass_kernel_spmd

    nc = _get_program()
    res = run_bass_kernel_spmd(nc, _in_maps(inputs), core_ids=list(range(8)))
    y = np.empty((B, S, D), dtype=np.float32)
    for c in range(8):
        b_, half = c // 2, c % 2
        y[b_, half * T:(half + 1) * T] = res.results[c]["out"]
    return y



# revision 3
# speedup vs baseline: 1.1800x; 1.1800x over previous
"""Trainium2 Bass kernel for nn_EncoderBlock (B=4, S=1024, D=1024, H=16, DFF=4096).

Sharding: 8 cores = 4 batches x 2 sequence-halves; each core produces the
block output for its 512 "own" tokens. Attention needs K/V for the batch's
full sequence, so the K/V-stream projections run on all 1024 tokens on both
cores of a batch pair (duplicated) -- zero inter-core communication.  The
host rolls each core's copy of x so its own tokens are always rows [0, T).

Layouts: activations feature-major ([feature, token], features on SBUF
partitions) so weights are stationary matmul operands in natural [in, out]
layout. Matmuls in bf16 (f32 PSUM accumulation); weights are cast to bf16 and
prepacked on the host so every weight DMA is one large contiguous transfer.

Scheduling structure (v1):
- x arrives token-major over plain DMA and is transposed to feature-major on
  the PE (64 [128,128] transposes) -- much faster than DMA-xbar transposes,
  and it warms the HAM clock before the projection stream starts.
- One whole-kernel weight staging pool with a 32-slot rotating tag holds
  wk/wv/whv/wq/whk/whq/wo and finally the resident W2 (same 64KB/partition
  W2 needs anyway).  All weight DMAs are emitted upfront on the Sync queue;
  slot write-after-read deps stagger them so each weight streams in exactly
  one phase ahead of its use -- no phase starts on a cold DMA.
- Attention softmax normalization is software-pipelined one head behind: the
  denominator-broadcast matmul for head h-1 issues between the scores and AV
  matmuls of head h, so the PE queue never waits on the DVE reciprocal.
- FFN2 keeps W2 resident and accumulates per output tile (k-inner), so
  residual+LN+store of tile i overlap the matmuls of tile i+1.

Attention per head: scores key-major (s[k_tok, q_tok]); softmax is
unnormalized exp (scores ~N(0, 0.03) here, no max subtraction needed);
denominators come from an appended ones-column on the V stationary operand;
normalization multiplies the head output by a PE-broadcast reciprocal.
"""

import math
import numpy as np

B, S, D, H = 4, 1024, 1024, 16
HD = D // H
DFF = 4 * D
T = S // 2
P = 128
NT = T // P     # 4
NS = S // P     # 8
ND = D // P     # 8
NHP = H // 2    # 8
NF = DFF // P   # 32
EPS = 1e-5
SCL = 1.0 / math.sqrt(D)

_CACHE = {}


def _build():
    import concourse.mybir as mybir
    import concourse.tile as tile
    from concourse import bacc
    from concourse.masks import make_identity
    from contextlib import ExitStack

    F32 = mybir.dt.float32
    F32R = mybir.dt.float32r
    BF16 = mybir.dt.bfloat16
    AF = mybir.ActivationFunctionType
    OP = mybir.AluOpType

    nc = bacc.Bacc(None, target_bir_lowering=False, debug=False)

    with tile.TileContext(nc) as tc:
        es = ExitStack()
        dram = es.enter_context(tc.tile_pool(name="dram", bufs=1, space="DRAM"))

        def din(name, shape, dt=BF16):
            return dram.tile(shape, dt, kind="ExternalInput", name=name, uniquify=False)

        x_bf = din("x_bf", [S, D])            # batch's full sequence (rolled), bf16
        x_own = din("x_own", [T, D], F32)     # own tokens, f32 (residual)
        Wk = din("Wk", [D, D]); Wq = din("Wq", [D, D]); Wv = din("Wv", [D, D])
        Whq = din("Whq_p", [NHP, P, D])       # [hp, p, (c h' e)] prepacked
        Whk = din("Whk_p", [NHP, P, D])
        Whv = din("Whv_p", [ND, P, D])        # [c, p, (h e)] prepacked
        Wo = din("Wo", [D, D])
        W1 = din("W1_p", [8, D, 512])         # [blk, d, j] prepacked
        W2 = din("W2", [DFF, D])
        bk = din("bk", [D], F32); bq = din("bq", [D], F32); bv = din("bv", [D], F32)
        bhq = din("bhq", [H, HD], F32); bhk = din("bhk", [H, HD], F32)
        bhv = din("bhv", [H, HD], F32R)
        bo = din("bo", [D], F32R); b1 = din("b1", [DFF], F32); b2 = din("b2", [D], F32R)
        out = dram.tile([T, D], F32, kind="ExternalOutput", name="out", uniquify=False)

        # ---------------- constants / psum ----------------
        const = es.enter_context(tc.tile_pool(name="const", bufs=1))
        ident = const.tile([P, P], F32, name="ident")
        make_identity(nc, ident)
        identb = const.tile([P, P], BF16, name="identb")
        nc.vector.tensor_copy(identb[:], ident[:])
        ones_f32 = const.tile([P, 16], F32, name="ones_f32")
        nc.vector.memset(ones_f32[:], 1.0)
        onesf2 = const.tile([P, P], F32, name="onesf2")
        nc.vector.memset(onesf2[:], 1.0)
        ones_r = const.tile([P, P], F32R, name="ones_r")
        nc.scalar.copy(ones_r[:], onesf2[:])
        eps_t = const.tile([P, 1], F32, name="eps_t")
        nc.vector.memset(eps_t[:], EPS)

        bo_rt = const.tile([1, D], F32R, name="bo_rt")
        nc.gpsimd.dma_start(out=bo_rt[:], in_=bo[:].rearrange("(o d) -> o d", o=1))
        b2_rt = const.tile([1, D], F32R, name="b2_rt")
        nc.gpsimd.dma_start(out=b2_rt[:], in_=b2[:].rearrange("(o d) -> o d", o=1))
        bhv_rt = const.tile([1, D], F32R, name="bhv_rt")
        nc.gpsimd.dma_start(out=bhv_rt[:], in_=bhv[:].rearrange("(o h) e -> o (h e)", o=1))
        bo_r, b2_r, bhv_r = bo_rt[:], b2_rt[:], bhv_rt[:]

        def bias_cols(name, vec, ncols):
            t = const.tile([P, ncols], F32, name=name)
            nc.gpsimd.dma_start(out=t[:], in_=vec.rearrange("(m p) -> p m", p=P))
            return t

        bk_t = bias_cols("bk_t", bk[:], ND)
        bq_t = bias_cols("bq_t", bq[:], ND)
        bv_t = bias_cols("bv_t", bv[:], ND)
        bhq_t = bias_cols("bhq_t", bhq[:].rearrange("h e -> (h e)"), NHP)
        bhk_t = bias_cols("bhk_t", bhk[:].rearrange("h e -> (h e)"), NHP)
        b1_t = bias_cols("b1_t", b1[:], NF)

        ln_p = es.enter_context(tc.tile_pool(name="ln_p", bufs=3))
        psum = es.enter_context(tc.tile_pool(name="psum", bufs=1, space="PSUM"))

        def ps_tile(name, shape=(P, 512), tag="ps", bufs=4, dt=F32):
            return psum.tile(list(shape), dt, name=name, tag=tag, bufs=bufs)

        ev_i = [0]
        ev_dve_only = [False]

        def evict(dst, src, bias=None):
            """PSUM -> SBUF eviction: 2 of 3 on DVE, 1 of 3 on ACT."""
            i = ev_i[0]; ev_i[0] += 1
            if i % 3 == 2 and not ev_dve_only[0]:
                if bias is None:
                    nc.scalar.copy(dst, src)
                else:
                    nc.scalar.activation(dst, src, AF.Identity, bias=bias)
            else:
                if bias is None:
                    nc.vector.tensor_copy(dst, src)
                else:
                    nc.vector.tensor_scalar_add(dst, src, bias)

        # ------- whole-kernel weight staging pool: 32 rotating [P, D] slots ----
        wstage = es.enter_context(tc.tile_pool(name="wstage", bufs=1))

        def wtiles(name, w_dram, rows=True, n=ND):
            sb = []
            for k in range(n):
                wt = wstage.tile([P, D], BF16, name=f"w_{name}{k}", tag="w", bufs=32)
                src = w_dram[k * P:(k + 1) * P, :] if rows else w_dram[k]
                nc.sync.dma_start(out=wt[:], in_=src)
                sb.append(wt)
            return sb

        # emission (= Sync queue) order is use order; slot write-after-read
        # deps against the tile 32 allocations earlier stagger the stream.
        wk_sb = wtiles("wk", Wk)
        wv_sb = wtiles("wv", Wv)
        whv_sb = wtiles("whv", Whv, rows=False)
        wq_sb = wtiles("wq", Wq)
        whk_sb = wtiles("whk", Whk, rows=False)
        whq_sb = wtiles("whq", Whq, rows=False)

        # right-side persistent pools (bottom: longest-lived)
        posb = ExitStack()
        osb_pool = posb.enter_context(tc.tile_pool(name="osb_pool", bufs=1, side="right"))
        o_sb = [osb_pool.tile([P, T], BF16, name=f"o_sb{hp}") for hp in range(NHP)]
        pva = ExitStack()
        va_pool = pva.enter_context(tc.tile_pool(name="va_pool", bufs=1, side="right"))
        v_aug = [va_pool.tile([P, H * (HD + 1)], BF16, name=f"vaug{i}") for i in range(NS)]
        pkt = ExitStack()
        kt_pool = pkt.enter_context(tc.tile_pool(name="kt_pool", bufs=1, side="right"))
        k_t = [kt_pool.tile([P, S], BF16, name=f"kh_o{m}") for m in range(NHP)]
        pqt = ExitStack()
        qt_pool = pqt.enter_context(tc.tile_pool(name="qt_pool", bufs=1, side="right"))
        q_t = [qt_pool.tile([P, T], BF16, name=f"qh_o{m}") for m in range(NHP)]

        # left-side long-lived: ko/qo (read inside the attention loop)
        p_ko = ExitStack()
        ko_pool = p_ko.enter_context(tc.tile_pool(name="ko_pool", bufs=1))
        p_qo = ExitStack()
        qo_pool = p_qo.enter_context(tc.tile_pool(name="qo_pool", bufs=1))

        # ================= Phase A: load x token-major, transpose on PE ========
        pxf = ExitStack()
        xf_p = pxf.enter_context(tc.tile_pool(name="xf_p", bufs=1))
        xf_t = [xf_p.tile([P, S], BF16, name=f"xf_t{j}") for j in range(ND)]
        pxtm = ExitStack()
        xtm_p = pxtm.enter_context(tc.tile_pool(name="xtm_p", bufs=1))
        xtm = [xtm_p.tile([P, D], BF16, name=f"xtm{i}") for i in range(NS)]
        for i in range(NS):
            nc.scalar.dma_start(out=xtm[i][:], in_=x_bf[i * P:(i + 1) * P, :])
        for i in range(NS):
            for j in range(ND):
                tp = ps_tile(f"tp{i}_{j}", shape=(P, P), tag="ps", dt=BF16)
                nc.tensor.transpose(tp[:P, :P], xtm[i][:, j * P:(j + 1) * P], identb[:])
                evict(xf_t[j][:, i * P:(i + 1) * P], tp[:P, :P])
        pxtm.close()

        # =============== dense projection helper ===============
        def wproj(name, w_sb, n_tok, bias_col, pool_out):
            """Dense [D, D] projection, feature-major output (BF16)."""
            outs = [pool_out.tile([P, n_tok], BF16, name=f"{name}_o{m}") for m in range(ND)]
            for m in range(ND):
                for n in range(n_tok // 512):
                    ps = ps_tile(f"ps_{name}{m}_{n}")
                    for k in range(ND):
                        nc.tensor.matmul(ps[:], w_sb[k][:, m * P:(m + 1) * P],
                                         xf_t[k][:, n * 512:(n + 1) * 512],
                                         start=(k == 0), stop=(k == ND - 1))
                    evict(outs[m][:, n * 512:(n + 1) * 512], ps[:],
                          bias=bias_col[:, m:m + 1])
            return outs

        # =============== Phase B0: Q-stream outer (own tokens = cols [0,T)) ====
        ko_t = wproj("ko", wk_sb, T, bk_t, ko_pool)

        # =============== Phase B1: V stream -> v_aug ===============
        p_vo = ExitStack()
        vo_pool = p_vo.enter_context(tc.tile_pool(name="vo_pool", bufs=1))
        vo_t = wproj("vo", wv_sb, S, bv_t, vo_pool)

        for i in range(NS):
            for n in range(2):
                ps = ps_tile(f"vkm{i}_{n}")
                for k in range(ND):
                    nc.tensor.matmul(ps[:], vo_t[k][:, i * P:(i + 1) * P],
                                     whv_sb[k][:, n * 512:(n + 1) * 512],
                                     start=(k == 0), stop=False)
                nc.tensor.matmul(ps[:], ones_r[:1, 0:P], bhv_r[:, n * 512:(n + 1) * 512],
                                 start=False, stop=True)
                dst = v_aug[i][:].rearrange("p (h e) -> p h e", e=HD + 1)
                evict(dst[:, 8 * n:8 * (n + 1), 0:HD],
                      ps[:].rearrange("p (h e) -> p h e", e=HD))
            dst = v_aug[i][:].rearrange("p (h e) -> p h e", e=HD + 1)
            nc.vector.tensor_copy(dst[:, :, HD:HD + 1],
                                  ones_f32[:, 0:H].rearrange("p (h o) -> p h o", o=1))
        p_vo.close()

        # =============== Phase B2: K-stream outer (full sequence) =============
        qo_t = wproj("qo", wq_sb, S, bq_t, qo_pool)
        pxf.close()

        # ====== interleaved loop: per head pair, K/Q head proj + attention ======
        pc = ExitStack()
        pkm_p = pc.enter_context(tc.tile_pool(name="pkm", bufs=16))
        den_p = pc.enter_context(tc.tile_pool(name="den_p", bufs=3))
        ev_dve_only[0] = True

        x_tok = [None] * NT
        wo_sb = [None] * ND
        pending = [None, None, None]   # [head, den, ops] awaiting normalization

        def finish():
            """Normalize pending head: broadcast 1/den over HD rows, multiply."""
            h, den, ops = pending
            hp, hl = h // 2, (h % 2) * HD
            bc = ps_tile(f"bc{h}", shape=(HD, T), tag="bc", bufs=2)
            nc.tensor.matmul(bc[:], ones_r[:1, 0:HD], den[:], start=True, stop=True)
            bcs = den_p.tile([HD, T], F32, name=f"bcs{h}", tag="bcs")
            nc.vector.tensor_copy(bcs[:], bc[:])
            nc.vector.tensor_tensor(o_sb[hp][hl:hl + HD, :], ops[0:HD, :], bcs[:],
                                    op=OP.mult)

        for hp in range(NHP):
            # k_t[hp]: per-head K projection over the full sequence
            for n in range(2):
                ps = ps_tile(f"ps_kh{hp}_{n}")
                for k in range(ND):
                    nc.tensor.matmul(ps[:], whk_sb[hp][:, k * P:(k + 1) * P],
                                     qo_t[k][:, n * 512:(n + 1) * 512],
                                     start=(k == 0), stop=(k == ND - 1))
                evict(k_t[hp][:, n * 512:(n + 1) * 512], ps[:],
                      bias=bhk_t[:, hp:hp + 1])
            # q_t[hp]: per-head Q projection over own tokens
            ps = ps_tile(f"ps_qh{hp}")
            for k in range(ND):
                nc.tensor.matmul(ps[:], whq_sb[hp][:, k * P:(k + 1) * P], ko_t[k][:],
                                 start=(k == 0), stop=(k == ND - 1))
            evict(q_t[hp][:], ps[:], bias=bhq_t[:, hp:hp + 1])

            # prefetch Phase D inputs under the attention loop
            if hp == 2:
                for i in range(NT):
                    x_tok[i] = wstage.tile([P, D], F32, name=f"x_tok{i}",
                                           tag="xtok", bufs=NT)
                    nc.gpsimd.dma_start(out=x_tok[i][:],
                                        in_=x_own[i * P:(i + 1) * P, :])
            if hp == 4:
                for k in range(ND):
                    wo_sb[k] = wstage.tile([P, D], BF16, name=f"wo{k}",
                                           tag="w", bufs=32)
                    nc.sync.dma_start(out=wo_sb[k][:], in_=Wo[k * P:(k + 1) * P, :])

            # attention for the two heads of this pair
            for h in (2 * hp, 2 * hp + 1):
                hl = (h % 2) * HD
                p_km = []
                for i in range(NS):
                    ps = ps_tile(f"sc{h}_{i}")
                    nc.tensor.matmul(ps[:], k_t[hp][hl:hl + HD, i * P:(i + 1) * P],
                                     q_t[hp][hl:hl + HD, :], start=True, stop=True)
                    pk = pkm_p.tile([P, T], BF16, name=f"pkm{h}_{i}", tag="pkm")
                    nc.scalar.activation(pk[:], ps[:], AF.Exp, scale=SCL)
                    p_km.append(pk)
                # normalization of the previous head rides between scores and
                # AV so the PE never waits on the DVE reciprocal
                if pending[0] is not None:
                    finish()
                ops = ps_tile(f"ops{h}", shape=(HD + 1, T), tag="ops", bufs=2)
                for i in range(NS):
                    nc.tensor.matmul(ops[:], v_aug[i][:, h * (HD + 1):(h + 1) * (HD + 1)],
                                     p_km[i][:], start=(i == 0), stop=(i == NS - 1))
                den = den_p.tile([1, T], F32R, name=f"den{h}", tag="den")
                with nc.allow_low_precision(reason="f32r is 4-byte f32 storage"):
                    nc.vector.reciprocal(den[:], ops[HD:HD + 1, :])
                pending = [h, den, ops]
        finish()
        ev_dve_only[0] = False
        pc.close()
        pqt.close(); pkt.close(); pva.close()
        p_qo.close(); p_ko.close()

        # =============== Phase D: output proj + residual + LN1 ===============
        pr1 = ExitStack()
        r1_pool = pr1.enter_context(tc.tile_pool(name="r1_pool", bufs=1))
        r1 = [r1_pool.tile([P, D], F32, name=f"r1_{i}") for i in range(NT)]
        r1_t = [r1_pool.tile([P, T], BF16, name=f"r1t{j}") for j in range(ND)]
        pe1 = ExitStack()
        ht_pool = pe1.enter_context(tc.tile_pool(name="ht_pool", bufs=1))
        h_t = [ht_pool.tile([P, T], BF16, name=f"h_t{m}") for m in range(NF)]
        e1s = ExitStack()
        w1_p = e1s.enter_context(tc.tile_pool(name="w1_p", bufs=24))
        w1_first = []
        for k in range(ND):
            wt = w1_p.tile([P, 512], BF16, name=f"w1_0_{k}", tag="w1")
            nc.sync.dma_start(out=wt[:], in_=W1[0, k * P:(k + 1) * P, :])
            w1_first.append(wt)
        pd = ExitStack()
        pre_p = pd.enter_context(tc.tile_pool(name="pre_p", bufs=2))

        def layernorm(tag, i, pre, dst):
            """dst = LN(pre) along free dim (D=1024). pre: [P, D] f32 SBUF."""
            st = ln_p.tile([P, 12], F32, name=f"st{tag}{i}", tag="st")
            nc.vector.bn_stats(st[:, 0:6], pre[:, 0:512])
            nc.vector.bn_stats(st[:, 6:12], pre[:, 512:1024])
            ag = ln_p.tile([P, 2], F32, name=f"ag{tag}{i}", tag="ag")
            nc.vector.bn_aggr(ag[:], st[:].rearrange("p (n s) -> p n s", n=2))
            sd = ln_p.tile([P, 1], F32, name=f"sd{tag}{i}", tag="sd")
            nc.scalar.activation(sd[:], ag[:, 1:2], AF.Sqrt, bias=eps_t[:])
            rs = ln_p.tile([P, 1], F32, name=f"rs{tag}{i}", tag="rs")
            nc.vector.reciprocal(rs[:], sd[:])
            nc.vector.tensor_scalar(dst, pre[:], ag[:, 0:1], rs[:],
                                    op0=OP.subtract, op1=OP.mult)

        def d_proj(i):
            pre = pre_p.tile([P, D], F32, name=f"pre1_{i}", tag="pre1")
            for n in range(2):
                ps = ps_tile(f"at{i}_{n}")
                for k in range(ND):
                    nc.tensor.matmul(ps[:], o_sb[k][:, i * P:(i + 1) * P],
                                     wo_sb[k][:, n * 512:(n + 1) * 512],
                                     start=(k == 0), stop=False)
                nc.tensor.matmul(ps[:], ones_r[:1, 0:P], bo_r[:, n * 512:(n + 1) * 512],
                                 start=False, stop=True)
                nc.vector.tensor_tensor(pre[:, n * 512:(n + 1) * 512], ps[:],
                                        x_tok[i][:, n * 512:(n + 1) * 512], op=OP.add)
            layernorm("r", i, pre, r1[i][:])

        def d_transpose(i):
            for j in range(ND):
                tp = ps_tile(f"r1tp{j}_{i}", shape=(P, P), tag="ops", bufs=2)
                nc.tensor.transpose(tp[:P, :P], r1[i][:, j * P:(j + 1) * P], ident[:])
                nc.scalar.copy(r1_t[j][:, i * P:(i + 1) * P], tp[:P, :P])

        d_proj(0)
        d_proj(1)
        d_transpose(0)
        d_proj(2)
        d_transpose(1)
        d_proj(3)
        d_transpose(2)
        d_transpose(3)
        pd.close()
        posb.close()

        # =============== Phase E: FFN1 (stream W1, prefetch W2) ===============
        w2_sb = [None] * NF
        for blk in range(8):            # dff blocks of 512
            if blk == 0:
                w1_sb = w1_first
            else:
                w1_sb = []
                for k in range(ND):
                    wt = w1_p.tile([P, 512], BF16, name=f"w1_{blk}_{k}", tag="w1")
                    nc.sync.dma_start(out=wt[:], in_=W1[blk, k * P:(k + 1) * P, :])
                    w1_sb.append(wt)
            # interleave W2 prefetch (4 tiles per block) on the same queue;
            # W2 reuses the "w" staging slots freed by whk/whq/wo/wq
            for k in range(4 * blk, 4 * blk + 4):
                w2_sb[k] = wstage.tile([P, D], BF16, name=f"w2_{k}", tag="w", bufs=32)
                nc.sync.dma_start(out=w2_sb[k][:], in_=W2[k * P:(k + 1) * P, :])
            for mm in range(4):         # 128-chunks within the block
                m = blk * 4 + mm
                ps = ps_tile(f"ff1_{m}")
                for k in range(ND):
                    nc.tensor.matmul(ps[:], w1_sb[k][:, mm * P:(mm + 1) * P],
                                     r1_t[k][:], start=(k == 0), stop=(k == ND - 1))
                nc.scalar.activation(h_t[m][:], ps[:], AF.Gelu, bias=b1_t[:, m:m + 1])
        e1s.close()

        # =============== Phase E2: FFN2 per output tile (W2 resident) =========
        pout = ExitStack()
        out_p = pout.enter_context(tc.tile_pool(name="out_p", bufs=2))
        tags = [("ps", 4), ("ps", 4), ("ops", 2), ("bc", 2)]
        for i in range(NT):
            tag, bufs = tags[i]
            pss = [ps_tile(f"ff2_{i}_{n}", shape=(P, 512), tag=tag, bufs=bufs)
                   for n in range(2)]
            for k in range(NF):
                for n in range(2):
                    nc.tensor.matmul(pss[n][:], h_t[k][:, i * P:(i + 1) * P],
                                     w2_sb[k][:, n * 512:(n + 1) * 512],
                                     start=(k == 0), stop=False)
            pre = out_p.tile([P, D], F32, name=f"pre2_{i}", tag="pre2")
            for n in range(2):
                nc.tensor.matmul(pss[n][:], ones_r[:1, 0:P],
                                 b2_r[:, n * 512:(n + 1) * 512], start=False, stop=True)
                nc.vector.tensor_tensor(pre[:, n * 512:(n + 1) * 512], pss[n][:],
                                        r1[i][:, n * 512:(n + 1) * 512],
                                        op=OP.add)
            o_sb2 = out_p.tile([P, D], F32, name=f"osb2_{i}", tag="osb2")
            layernorm("o", i, pre, o_sb2[:])
            nc.gpsimd.dma_start(out=out[i * P:(i + 1) * P, :], in_=o_sb2[:])
        pout.close()
        pe1.close()
        pr1.close()

        es.close()
    nc.compile()
    return nc


def _get_program():
    if "nc" not in _CACHE:
        _CACHE["nc"] = _build()
    return _CACHE["nc"]


def _prepack(inputs):
    """Cast weights to bf16 and prepack into SBUF tile layouts."""
    import ml_dtypes
    bf16 = ml_dtypes.bfloat16

    def b(a):
        return np.ascontiguousarray(np.asarray(a, dtype=np.float32).astype(bf16))

    Whq = np.asarray(inputs["Whq"], dtype=np.float32)
    Whk = np.asarray(inputs["Whk"], dtype=np.float32)
    Whv = np.asarray(inputs["Whv"], dtype=np.float32)
    W1 = np.asarray(inputs["W1"], dtype=np.float32)
    # [hp, p, (c h' e)]: Whq_p[hp, p, c*128+h'*64+e] = Whq[2hp+h', c*128+p, e]
    whq_p = b(Whq.reshape(NHP, 2, ND, P, HD).transpose(0, 3, 2, 1, 4).reshape(NHP, P, D))
    whk_p = b(Whk.reshape(NHP, 2, ND, P, HD).transpose(0, 3, 2, 1, 4).reshape(NHP, P, D))
    # [c, p, (h e)]: Whv_p[c, p, h*64+e] = Whv[h, c*128+p, e]
    whv_p = b(Whv.reshape(H, ND, P, HD).transpose(1, 2, 0, 3).reshape(ND, P, D))
    # [blk, d, j]
    w1_p = b(W1.reshape(D, 8, 512).transpose(1, 0, 2))
    f32 = lambda n: np.ascontiguousarray(inputs[n], dtype=np.float32)
    return {
        "Wk": b(inputs["Wk"]), "Wq": b(inputs["Wq"]), "Wv": b(inputs["Wv"]),
        "Wo": b(inputs["Wo"]), "W2": b(inputs["W2"]),
        "Whq_p": whq_p, "Whk_p": whk_p, "Whv_p": whv_p, "W1_p": w1_p,
        "bk": f32("bk"), "bq": f32("bq"), "bv": f32("bv"),
        "bhq": f32("bhq"), "bhk": f32("bhk"), "bhv": f32("bhv"),
        "bo": f32("bo"), "b1": f32("b1"), "b2": f32("b2"),
    }


def _in_maps(inputs):
    import ml_dtypes
    x = np.ascontiguousarray(inputs["x"], dtype=np.float32)
    x_bf = x.astype(ml_dtypes.bfloat16)
    wmap = _prepack(inputs)
    in_maps = []
    for c in range(8):
        b_, half = c // 2, c % 2
        m = dict(wmap)
        # roll the sequence so this core's own half occupies rows [0, T);
        # attention is permutation-invariant over keys, so only the query
        # (= own token) rows need a consistent convention.
        m["x_bf"] = np.ascontiguousarray(np.roll(x_bf[b_], -half * T, axis=0))
        m["x_own"] = x[b_, half * T:(half + 1) * T]
        in_maps.append(m)
    return in_maps


def kernel(**inputs):
    from concourse.bass_utils import run_bass_kernel_spmd

    nc = _get_program()
    res = run_bass_kernel_spmd(nc, _in_maps(inputs), core_ids=list(range(8)))
    y = np.empty((B, S, D), dtype=np.float32)
    for c in range(8):
        b_, half = c // 2, c % 2
        y[b_, half * T:(half + 1) * T] = res.results[c]["out"]
    return y


# revision 7
# speedup vs baseline: 1.2844x; 1.0885x over previous
"""Trainium2 Bass kernel for nn_EncoderBlock (B=4, S=1024, D=1024, H=16, DFF=4096).

Sharding: 8 cores = 4 batches x 2 sequence-halves; each core produces the
block output for its 512 "own" tokens. Attention needs K/V for the batch's
full sequence, so the K/V-stream projections run on all 1024 tokens on both
cores of a batch pair (duplicated) -- zero inter-core communication.  The
host rolls each core's copy of x so its own tokens are always rows [0, T).

Layouts: activations feature-major ([feature, token], features on SBUF
partitions) so weights are stationary matmul operands in natural [in, out]
layout. Matmuls in bf16 (f32 PSUM accumulation); weights are cast to bf16 and
prepacked on the host so every weight DMA is one large contiguous transfer.

Scheduling structure (v1):
- x arrives token-major over plain DMA and is transposed to feature-major on
  the PE (64 [128,128] transposes) -- much faster than DMA-xbar transposes,
  and it warms the HAM clock before the projection stream starts.
- One whole-kernel weight staging pool with a 32-slot rotating tag holds
  wk/wv/whv/wq/whk/whq/wo and finally the resident W2 (same 64KB/partition
  W2 needs anyway).  All weight DMAs are emitted upfront on the Sync queue;
  slot write-after-read deps stagger them so each weight streams in exactly
  one phase ahead of its use -- no phase starts on a cold DMA.
- Attention softmax normalization is software-pipelined one head behind: the
  denominator-broadcast matmul for head h-1 issues between the scores and AV
  matmuls of head h, so the PE queue never waits on the DVE reciprocal.
- FFN2 keeps W2 resident and accumulates per output tile (k-inner), so
  residual+LN+store of tile i overlap the matmuls of tile i+1.

Attention per head: scores key-major (s[k_tok, q_tok]); softmax is
unnormalized exp (scores ~N(0, 0.03) here, no max subtraction needed);
denominators come from an appended ones-column on the V stationary operand;
normalization multiplies the head output by a PE-broadcast reciprocal.
"""

import math
import numpy as np

B, S, D, H = 4, 1024, 1024, 16
HD = D // H
DFF = 4 * D
T = S // 2
P = 128
NT = T // P     # 4
NS = S // P     # 8
ND = D // P     # 8
NHP = H // 2    # 8
NF = DFF // P   # 32
EPS = 1e-5
SCL = 1.0 / math.sqrt(D)

_CACHE = {}


def _build():
    import concourse.mybir as mybir
    import concourse.tile as tile
    from concourse import bacc
    from concourse.masks import make_identity
    from contextlib import ExitStack

    F32 = mybir.dt.float32
    F32R = mybir.dt.float32r
    BF16 = mybir.dt.bfloat16
    AF = mybir.ActivationFunctionType
    OP = mybir.AluOpType

    nc = bacc.Bacc(None, target_bir_lowering=False, debug=False)

    with tile.TileContext(nc) as tc:
        es = ExitStack()
        dram = es.enter_context(tc.tile_pool(name="dram", bufs=1, space="DRAM"))

        def din(name, shape, dt=BF16):
            return dram.tile(shape, dt, kind="ExternalInput", name=name, uniquify=False)

        x_bf = din("x_bf", [S, D])            # batch's full sequence (rolled), bf16
        x_own = din("x_own", [T, D], F32)     # own tokens, f32 (residual)
        Wk = din("Wk", [D, D]); Wq = din("Wq", [D, D]); Wv = din("Wv", [D, D])
        Whq = din("Whq_p", [NHP, P, D])       # [hp, p, (c h' e)] prepacked
        Whk = din("Whk_p", [NHP, P, D])
        Whv = din("Whv_p", [ND, P, D])        # [c, p, (h e)] prepacked
        Wo = din("Wo", [D, D])
        W1 = din("W1_p", [8, D, 512])         # [blk, d, j] prepacked
        W2 = din("W2", [DFF, D])
        bk = din("bk", [D], F32); bq = din("bq", [D], F32); bv = din("bv", [D], F32)
        bhq = din("bhq", [H, HD], F32); bhk = din("bhk", [H, HD], F32)
        bhv = din("bhv", [H, HD], F32R)
        bo = din("bo", [D], F32R); b1 = din("b1", [DFF], F32); b2 = din("b2", [D], F32R)
        out = dram.tile([T, D], F32, kind="ExternalOutput", name="out", uniquify=False)

        # ---------------- constants / psum ----------------
        const = es.enter_context(tc.tile_pool(name="const", bufs=1))
        ident = const.tile([P, P], F32, name="ident")
        make_identity(nc, ident)
        identb = const.tile([P, P], BF16, name="identb")
        nc.vector.tensor_copy(identb[:], ident[:])
        ones_f32 = const.tile([P, 16], F32, name="ones_f32")
        nc.vector.memset(ones_f32[:], 1.0)
        onesf2 = const.tile([P, P], F32, name="onesf2")
        nc.vector.memset(onesf2[:], 1.0)
        ones_r = const.tile([P, P], F32R, name="ones_r")
        nc.scalar.copy(ones_r[:], onesf2[:])
        eps_t = const.tile([P, 1], F32, name="eps_t")
        nc.vector.memset(eps_t[:], EPS)

        bo_rt = const.tile([1, D], F32R, name="bo_rt")
        nc.gpsimd.dma_start(out=bo_rt[:], in_=bo[:].rearrange("(o d) -> o d", o=1))
        b2_rt = const.tile([1, D], F32R, name="b2_rt")
        nc.gpsimd.dma_start(out=b2_rt[:], in_=b2[:].rearrange("(o d) -> o d", o=1))
        bhv_rt = const.tile([1, D], F32R, name="bhv_rt")
        nc.gpsimd.dma_start(out=bhv_rt[:], in_=bhv[:].rearrange("(o h) e -> o (h e)", o=1))
        bo_r, b2_r, bhv_r = bo_rt[:], b2_rt[:], bhv_rt[:]

        def bias_cols(name, vec, ncols):
            t = const.tile([P, ncols], F32, name=name)
            nc.gpsimd.dma_start(out=t[:], in_=vec.rearrange("(m p) -> p m", p=P))
            return t

        bk_t = bias_cols("bk_t", bk[:], ND)
        bq_t = bias_cols("bq_t", bq[:], ND)
        bv_t = bias_cols("bv_t", bv[:], ND)
        bhq_t = bias_cols("bhq_t", bhq[:].rearrange("h e -> (h e)"), NHP)
        bhk_t = bias_cols("bhk_t", bhk[:].rearrange("h e -> (h e)"), NHP)
        b1_t = bias_cols("b1_t", b1[:], NF)

        ln_p = es.enter_context(tc.tile_pool(name="ln_p", bufs=3))
        psum = es.enter_context(tc.tile_pool(name="psum", bufs=1, space="PSUM"))

        def ps_tile(name, shape=(P, 512), tag="ps", bufs=4, dt=F32):
            return psum.tile(list(shape), dt, name=name, tag=tag, bufs=bufs)

        ev_i = [0]
        ev_dve_only = [False]

        def evict(dst, src, bias=None):
            """PSUM -> SBUF eviction: 2 of 3 on DVE, 1 of 3 on ACT."""
            i = ev_i[0]; ev_i[0] += 1
            if i % 3 == 2 and not ev_dve_only[0]:
                if bias is None:
                    nc.scalar.copy(dst, src)
                else:
                    nc.scalar.activation(dst, src, AF.Identity, bias=bias)
            else:
                if bias is None:
                    nc.vector.tensor_copy(dst, src)
                else:
                    nc.vector.tensor_scalar_add(dst, src, bias)

        # ------- whole-kernel weight staging pool: 32 rotating [P, D] slots ----
        wstage = es.enter_context(tc.tile_pool(name="wstage", bufs=1))

        def wtiles(name, w_dram, rows=True, n=ND):
            sb = []
            for k in range(n):
                wt = wstage.tile([P, D], BF16, name=f"w_{name}{k}", tag="w", bufs=32)
                src = w_dram[k * P:(k + 1) * P, :] if rows else w_dram[k]
                nc.sync.dma_start(out=wt[:], in_=src)
                sb.append(wt)
            return sb

        # emission (= Sync queue) order is use order; slot write-after-read
        # deps against the tile 32 allocations earlier stagger the stream.
        wk_sb = wtiles("wk", Wk)
        wv_sb = wtiles("wv", Wv)
        whv_sb = wtiles("whv", Whv, rows=False)
        wq_sb = wtiles("wq", Wq)
        whk_sb = wtiles("whk", Whk, rows=False)
        whq_sb = wtiles("whq", Whq, rows=False)

        # right-side persistent pools (bottom: longest-lived)
        posb = ExitStack()
        osb_pool = posb.enter_context(tc.tile_pool(name="osb_pool", bufs=1, side="right"))
        o_sb = [osb_pool.tile([P, T], BF16, name=f"o_sb{hp}") for hp in range(NHP)]
        pva = ExitStack()
        va_pool = pva.enter_context(tc.tile_pool(name="va_pool", bufs=1, side="right"))
        v_aug = [va_pool.tile([P, H * (HD + 1)], BF16, name=f"vaug{i}") for i in range(NS)]
        pkt = ExitStack()
        kt_pool = pkt.enter_context(tc.tile_pool(name="kt_pool", bufs=1, side="right"))
        k_t = [kt_pool.tile([P, S], BF16, name=f"kh_o{m}") for m in range(NHP)]
        pqt = ExitStack()
        qt_pool = pqt.enter_context(tc.tile_pool(name="qt_pool", bufs=1, side="right"))
        q_t = [qt_pool.tile([P, T], BF16, name=f"qh_o{m}") for m in range(NHP)]

        # left-side long-lived: ko/qo (read inside the attention loop)
        p_ko = ExitStack()
        ko_pool = p_ko.enter_context(tc.tile_pool(name="ko_pool", bufs=1))
        p_qo = ExitStack()
        qo_pool = p_qo.enter_context(tc.tile_pool(name="qo_pool", bufs=1))

        # ================= Phase A: load x token-major, transpose on PE ========
        pxf = ExitStack()
        xf_p = pxf.enter_context(tc.tile_pool(name="xf_p", bufs=1))
        xf_t = [xf_p.tile([P, S], BF16, name=f"xf_t{j}") for j in range(ND)]
        pxtm = ExitStack()
        xtm_p = pxtm.enter_context(tc.tile_pool(name="xtm_p", bufs=1))
        xtm = [xtm_p.tile([P, D], BF16, name=f"xtm{i}") for i in range(NS)]
        for i in range(NS):
            nc.scalar.dma_start(out=xtm[i][:], in_=x_bf[i * P:(i + 1) * P, :])

        def transpose_x(i_range):
            for i in i_range:
                for j in range(ND):
                    tp = ps_tile(f"tp{i}_{j}", shape=(P, P), tag="ps", dt=BF16)
                    nc.tensor.transpose(tp[:P, :P], xtm[i][:, j * P:(j + 1) * P],
                                        identb[:])
                    evict(xf_t[j][:, i * P:(i + 1) * P], tp[:P, :P])

        # own half first: Phase B0 only needs token columns [0, T)
        transpose_x(range(NT))

        # =============== dense projection helper ===============
        def wproj(name, w_sb, n_tok, bias_col, pool_out):
            """Dense [D, D] projection, feature-major output (BF16)."""
            outs = [pool_out.tile([P, n_tok], BF16, name=f"{name}_o{m}") for m in range(ND)]
            for m in range(ND):
                for n in range(n_tok // 512):
                    ps = ps_tile(f"ps_{name}{m}_{n}")
                    for k in range(ND):
                        nc.tensor.matmul(ps[:], w_sb[k][:, m * P:(m + 1) * P],
                                         xf_t[k][:, n * 512:(n + 1) * 512],
                                         start=(k == 0), stop=(k == ND - 1))
                    evict(outs[m][:, n * 512:(n + 1) * 512], ps[:],
                          bias=bias_col[:, m:m + 1])
            return outs

        # =============== Phase B0: Q-stream outer (own tokens = cols [0,T)) ====
        ko_t = wproj("ko", wk_sb, T, bk_t, ko_pool)
        transpose_x(range(NT, NS))      # other half, needed from B1 on
        pxtm.close()

        # =============== Phase B1: V stream -> v_aug ===============
        p_vo = ExitStack()
        vo_pool = p_vo.enter_context(tc.tile_pool(name="vo_pool", bufs=1))
        vo_t = wproj("vo", wv_sb, S, bv_t, vo_pool)

        for i in range(NS):
            for n in range(2):
                ps = ps_tile(f"vkm{i}_{n}")
                for k in range(ND):
                    nc.tensor.matmul(ps[:], vo_t[k][:, i * P:(i + 1) * P],
                                     whv_sb[k][:, n * 512:(n + 1) * 512],
                                     start=(k == 0), stop=False)
                nc.tensor.matmul(ps[:], ones_r[:1, 0:P], bhv_r[:, n * 512:(n + 1) * 512],
                                 start=False, stop=True)
                dst = v_aug[i][:].rearrange("p (h e) -> p h e", e=HD + 1)
                evict(dst[:, 8 * n:8 * (n + 1), 0:HD],
                      ps[:].rearrange("p (h e) -> p h e", e=HD))
            dst = v_aug[i][:].rearrange("p (h e) -> p h e", e=HD + 1)
            nc.vector.tensor_copy(dst[:, :, HD:HD + 1],
                                  ones_f32[:, 0:H].rearrange("p (h o) -> p h o", o=1))
        p_vo.close()

        # =============== Phase B2: K-stream outer (full sequence) =============
        qo_t = wproj("qo", wq_sb, S, bq_t, qo_pool)
        pxf.close()

        # ====== interleaved loop: per head pair, K/Q head proj + attention ======
        pc = ExitStack()
        pkm_p = pc.enter_context(tc.tile_pool(name="pkm", bufs=16))
        den_p = pc.enter_context(tc.tile_pool(name="den_p", bufs=3))
        ev_dve_only[0] = True

        x_tok = [None] * NT
        wo_sb = [None] * ND
        pending = [None, None, None]   # [head, den-row, ops] awaiting normalization

        def finish():
            """Normalize pending head: PE-broadcast the raw denominator over HD
            rows, then a single full-width DVE reciprocal (which is also the
            PSUM eviction), then multiply.  The PE only waits on the cheap
            den-row copy, never on the slow iterative reciprocal."""
            h, den, ops = pending
            hp, hl = h // 2, (h % 2) * HD
            bc = ps_tile(f"bc{h}", shape=(HD, T), tag="bc", bufs=2)
            nc.tensor.matmul(bc[:], ones_r[:1, 0:HD], den[:], start=True, stop=True)
            bcs = den_p.tile([HD, T], F32, name=f"bcs{h}", tag="bcs")
            nc.vector.reciprocal(bcs[:], bc[:])
            nc.vector.tensor_tensor(o_sb[hp][hl:hl + HD, :], ops[0:HD, :], bcs[:],
                                    op=OP.mult)

        for hp in range(NHP):
            # k_t[hp]: per-head K projection over the full sequence
            for n in range(2):
                ps = ps_tile(f"ps_kh{hp}_{n}")
                for k in range(ND):
                    nc.tensor.matmul(ps[:], whk_sb[hp][:, k * P:(k + 1) * P],
                                     qo_t[k][:, n * 512:(n + 1) * 512],
                                     start=(k == 0), stop=(k == ND - 1))
                evict(k_t[hp][:, n * 512:(n + 1) * 512], ps[:],
                      bias=bhk_t[:, hp:hp + 1])
            # q_t[hp]: per-head Q projection over own tokens
            ps = ps_tile(f"ps_qh{hp}")
            for k in range(ND):
                nc.tensor.matmul(ps[:], whq_sb[hp][:, k * P:(k + 1) * P], ko_t[k][:],
                                 start=(k == 0), stop=(k == ND - 1))
            evict(q_t[hp][:], ps[:], bias=bhq_t[:, hp:hp + 1])

            # prefetch Phase D inputs under the attention loop
            if hp == 2:
                for i in range(NT):
                    x_tok[i] = wstage.tile([P, D], F32, name=f"x_tok{i}",
                                           tag="xtok", bufs=NT)
                    nc.gpsimd.dma_start(out=x_tok[i][:],
                                        in_=x_own[i * P:(i + 1) * P, :])
            if hp == 4:
                for k in range(ND):
                    wo_sb[k] = wstage.tile([P, D], BF16, name=f"wo{k}",
                                           tag="w", bufs=32)
                    nc.sync.dma_start(out=wo_sb[k][:], in_=Wo[k * P:(k + 1) * P, :])

            # attention for the two heads of this pair
            for h in (2 * hp, 2 * hp + 1):
                hl = (h % 2) * HD
                p_km = []
                for i in range(NS):
                    ps = ps_tile(f"sc{h}_{i}")
                    nc.tensor.matmul(ps[:], k_t[hp][hl:hl + HD, i * P:(i + 1) * P],
                                     q_t[hp][hl:hl + HD, :], start=True, stop=True)
                    pk = pkm_p.tile([P, T], BF16, name=f"pkm{h}_{i}", tag="pkm")
                    nc.scalar.activation(pk[:], ps[:], AF.Exp, scale=SCL)
                    p_km.append(pk)
                # normalization of the previous head rides between scores and
                # AV so the PE never waits on the DVE reciprocal
                if pending[0] is not None:
                    finish()
                ops = ps_tile(f"ops{h}", shape=(HD + 1, T), tag="ops", bufs=2)
                for i in range(NS):
                    nc.tensor.matmul(ops[:], v_aug[i][:, h * (HD + 1):(h + 1) * (HD + 1)],
                                     p_km[i][:], start=(i == 0), stop=(i == NS - 1))
                den = den_p.tile([1, T], F32R, name=f"den{h}", tag="den")
                with nc.allow_low_precision(reason="f32r is 4-byte f32 storage"):
                    nc.vector.tensor_copy(den[:], ops[HD:HD + 1, :])
                pending = [h, den, ops]
        finish()
        ev_dve_only[0] = False
        pc.close()
        pqt.close(); pkt.close(); pva.close()
        p_qo.close(); p_ko.close()

        # =============== Phase D: output proj + residual + LN1 ===============
        pr1 = ExitStack()
        r1_pool = pr1.enter_context(tc.tile_pool(name="r1_pool", bufs=1))
        r1 = [r1_pool.tile([P, D], F32, name=f"r1_{i}") for i in range(NT)]
        r1_t = [r1_pool.tile([P, T], BF16, name=f"r1t{j}") for j in range(ND)]
        pe1 = ExitStack()
        ht_pool = pe1.enter_context(tc.tile_pool(name="ht_pool", bufs=1))
        h_t = [ht_pool.tile([P, T], BF16, name=f"h_t{m}") for m in range(NF)]
        e1s = ExitStack()
        w1_p = e1s.enter_context(tc.tile_pool(name="w1_p", bufs=24))
        w1_first = []
        for k in range(ND):
            wt = w1_p.tile([P, 512], BF16, name=f"w1_0_{k}", tag="w1")
            nc.sync.dma_start(out=wt[:], in_=W1[0, k * P:(k + 1) * P, :])
            w1_first.append(wt)
        pd = ExitStack()
        pre_p = pd.enter_context(tc.tile_pool(name="pre_p", bufs=2))

        def layernorm(tag, i, pre, dst):
            """dst = LN(pre) along free dim (D=1024). pre: [P, D] f32 SBUF."""
            st = ln_p.tile([P, 12], F32, name=f"st{tag}{i}", tag="st")
            nc.vector.bn_stats(st[:, 0:6], pre[:, 0:512])
            nc.vector.bn_stats(st[:, 6:12], pre[:, 512:1024])
            ag = ln_p.tile([P, 2], F32, name=f"ag{tag}{i}", tag="ag")
            nc.vector.bn_aggr(ag[:], st[:].rearrange("p (n s) -> p n s", n=2))
            sd = ln_p.tile([P, 1], F32, name=f"sd{tag}{i}", tag="sd")
            nc.scalar.activation(sd[:], ag[:, 1:2], AF.Sqrt, bias=eps_t[:])
            rs = ln_p.tile([P, 1], F32, name=f"rs{tag}{i}", tag="rs")
            nc.vector.reciprocal(rs[:], sd[:])
            nc.vector.tensor_scalar(dst, pre[:], ag[:, 0:1], rs[:],
                                    op0=OP.subtract, op1=OP.mult)

        def d_proj(i):
            pre = pre_p.tile([P, D], F32, name=f"pre1_{i}", tag="pre1")
            for n in range(2):
                ps = ps_tile(f"at{i}_{n}")
                for k in range(ND):
                    nc.tensor.matmul(ps[:], o_sb[k][:, i * P:(i + 1) * P],
                                     wo_sb[k][:, n * 512:(n + 1) * 512],
                                     start=(k == 0), stop=False)
                nc.tensor.matmul(ps[:], ones_r[:1, 0:P], bo_r[:, n * 512:(n + 1) * 512],
                                 start=False, stop=True)
                nc.vector.tensor_tensor(pre[:, n * 512:(n + 1) * 512], ps[:],
                                        x_tok[i][:, n * 512:(n + 1) * 512], op=OP.add)
            layernorm("r", i, pre, r1[i][:])

        def d_transpose(i):
            for j in range(ND):
                tp = ps_tile(f"r1tp{j}_{i}", shape=(P, P), tag="ops", bufs=2)
                nc.tensor.transpose(tp[:P, :P], r1[i][:, j * P:(j + 1) * P], ident[:])
                nc.scalar.copy(r1_t[j][:, i * P:(i + 1) * P], tp[:P, :P])

        d_proj(0)
        d_proj(1)
        d_transpose(0)
        d_proj(2)
        d_transpose(1)
        d_proj(3)
        d_transpose(2)
        d_transpose(3)
        pd.close()
        posb.close()

        # =============== Phase E: FFN1 (stream W1, prefetch W2) ===============
        w2_sb = [None] * NF
        for blk in range(8):            # dff blocks of 512
            if blk == 0:
                w1_sb = w1_first
            else:
                w1_sb = []
                for k in range(ND):
                    wt = w1_p.tile([P, 512], BF16, name=f"w1_{blk}_{k}", tag="w1")
                    nc.sync.dma_start(out=wt[:], in_=W1[blk, k * P:(k + 1) * P, :])
                    w1_sb.append(wt)
            # interleave W2 prefetch (4 tiles per block) on the same queue;
            # W2 reuses the "w" staging slots freed by whk/whq/wo/wq
            for k in range(4 * blk, 4 * blk + 4):
                w2_sb[k] = wstage.tile([P, D], BF16, name=f"w2_{k}", tag="w", bufs=32)
                nc.sync.dma_start(out=w2_sb[k][:], in_=W2[k * P:(k + 1) * P, :])
            for mm in range(4):         # 128-chunks within the block
                m = blk * 4 + mm
                ps = ps_tile(f"ff1_{m}")
                for k in range(ND):
                    nc.tensor.matmul(ps[:], w1_sb[k][:, mm * P:(mm + 1) * P],
                                     r1_t[k][:], start=(k == 0), stop=(k == ND - 1))
                nc.scalar.activation(h_t[m][:], ps[:], AF.Gelu, bias=b1_t[:, m:m + 1])
        e1s.close()

        # =============== Phase E2: FFN2 per output tile (W2 resident) =========
        pout = ExitStack()
        out_p = pout.enter_context(tc.tile_pool(name="out_p", bufs=2))
        tags = [("ps", 4), ("ps", 4), ("ops", 2), ("bc", 2)]
        for i in range(NT):
            tag, bufs = tags[i]
            pss = [ps_tile(f"ff2_{i}_{n}", shape=(P, 512), tag=tag, bufs=bufs)
                   for n in range(2)]
            for k in range(NF):
                for n in range(2):
                    nc.tensor.matmul(pss[n][:], h_t[k][:, i * P:(i + 1) * P],
                                     w2_sb[k][:, n * 512:(n + 1) * 512],
                                     start=(k == 0), stop=False)
            pre = out_p.tile([P, D], F32, name=f"pre2_{i}", tag="pre2")
            for n in range(2):
                nc.tensor.matmul(pss[n][:], ones_r[:1, 0:P],
                                 b2_r[:, n * 512:(n + 1) * 512], start=False, stop=True)
                nc.vector.tensor_tensor(pre[:, n * 512:(n + 1) * 512], pss[n][:],
                                        r1[i][:, n * 512:(n + 1) * 512],
                                        op=OP.add)
            o_sb2 = out_p.tile([P, D], F32, name=f"osb2_{i}", tag="osb2")
            layernorm("o", i, pre, o_sb2[:])
            nc.gpsimd.dma_start(out=out[i * P:(i + 1) * P, :], in_=o_sb2[:])
        pout.close()
        pe1.close()
        pr1.close()

        es.close()
    nc.compile()
    return nc


def _get_program():
    if "nc" not in _CACHE:
        _CACHE["nc"] = _build()
    return _CACHE["nc"]


def _prepack(inputs):
    """Cast weights to bf16 and prepack into SBUF tile layouts."""
    import ml_dtypes
    bf16 = ml_dtypes.bfloat16

    def b(a):
        return np.ascontiguousarray(np.asarray(a, dtype=np.float32).astype(bf16))

    Whq = np.asarray(inputs["Whq"], dtype=np.float32)
    Whk = np.asarray(inputs["Whk"], dtype=np.float32)
    Whv = np.asarray(inputs["Whv"], dtype=np.float32)
    W1 = np.asarray(inputs["W1"], dtype=np.float32)
    # [hp, p, (c h' e)]: Whq_p[hp, p, c*128+h'*64+e] = Whq[2hp+h', c*128+p, e]
    whq_p = b(Whq.reshape(NHP, 2, ND, P, HD).transpose(0, 3, 2, 1, 4).reshape(NHP, P, D))
    whk_p = b(Whk.reshape(NHP, 2, ND, P, HD).transpose(0, 3, 2, 1, 4).reshape(NHP, P, D))
    # [c, p, (h e)]: Whv_p[c, p, h*64+e] = Whv[h, c*128+p, e]
    whv_p = b(Whv.reshape(H, ND, P, HD).transpose(1, 2, 0, 3).reshape(ND, P, D))
    # [blk, d, j]
    w1_p = b(W1.reshape(D, 8, 512).transpose(1, 0, 2))
    f32 = lambda n: np.ascontiguousarray(inputs[n], dtype=np.float32)
    return {
        "Wk": b(inputs["Wk"]), "Wq": b(inputs["Wq"]), "Wv": b(inputs["Wv"]),
        "Wo": b(inputs["Wo"]), "W2": b(inputs["W2"]),
        "Whq_p": whq_p, "Whk_p": whk_p, "Whv_p": whv_p, "W1_p": w1_p,
        "bk": f32("bk"), "bq": f32("bq"), "bv": f32("bv"),
        "bhq": f32("bhq"), "bhk": f32("bhk"), "bhv": f32("bhv"),
        "bo": f32("bo"), "b1": f32("b1"), "b2": f32("b2"),
    }


def _in_maps(inputs):
    import ml_dtypes
    x = np.ascontiguousarray(inputs["x"], dtype=np.float32)
    x_bf = x.astype(ml_dtypes.bfloat16)
    wmap = _prepack(inputs)
    in_maps = []
    for c in range(8):
        b_, half = c // 2, c % 2
        m = dict(wmap)
        # roll the sequence so this core's own half occupies rows [0, T);
        # attention is permutation-invariant over keys, so only the query
        # (= own token) rows need a consistent convention.
        m["x_bf"] = np.ascontiguousarray(np.roll(x_bf[b_], -half * T, axis=0))
        m["x_own"] = x[b_, half * T:(half + 1) * T]
        in_maps.append(m)
    return in_maps


def kernel(**inputs):
    from concourse.bass_utils import run_bass_kernel_spmd

    nc = _get_program()
    res = run_bass_kernel_spmd(nc, _in_maps(inputs), core_ids=list(range(8)))
    y = np.empty((B, S, D), dtype=np.float32)
    for c in range(8):
        b_, half = c // 2, c % 2
        y[b_, half * T:(half + 1) * T] = res.results[c]["out"]
    return y


# revision 8
# speedup vs baseline: 1.7732x; 1.3805x over previous
"""Trainium2 Bass kernel for nn_EncoderBlock — fp8 (e4m3) DoubleRow variant.

Same schedule as kernel.py v1.5 (PE x-transposes, staged weight prefetch,
software-pipelined softmax normalization, per-tile FFN2 with resident W2),
with every large GEMM converted to fp8e4 DoubleRow matmuls: contraction of
256 per instruction at 2 cols/cycle — half the PE streaming time of bf16.

fp8 layouts: activations are stored as "pair tiles" [P, 2*N]: plane i at
columns [i*N, (i+1)*N) holds feature chunk 2c+i of pair c, matching the
[P, 2, N] access-pattern DoubleRow expects (contraction row = 256c+128i+p).
Weights are host-prepacked into the same pairing.

Precision notes: all fp8 paths carry ~2-3% RMS relative error, but they only
feed (a) attention, whose output is a small (~0.04 std) additive term on the
unit-variance residual, and (b) the FFN, whose output (~0.27 std) meets the
residual stream before a LayerNorm; the end-to-end max error stays well
under the 2e-2 gate.  Scores (contraction 64, no DoubleRow win) stay bf16.
Scaling: attention head outputs are scaled x16 (via the 1/16 broadcast
constant) and Wo x2 so both operands sit in e4m3's normal range; the
resulting x32 on the pre-LN1 sum is cancelled by passing 32*x_own and 32*bo
(LayerNorm is scale-invariant).
"""

import math
import numpy as np

B, S, D, H = 4, 1024, 1024, 16
HD = D // H
DFF = 4 * D
T = S // 2
P = 128
NT = T // P     # 4
NS = S // P     # 8
ND = D // P     # 8
NHP = H // 2    # 8
NF = DFF // P   # 32
NC = D // 256   # 4 double-contraction chunks
EPS = 1e-5
SCL = 1.0 / math.sqrt(D)
OSC = 16.0      # attention output scale (folded: x16 o, x2 Wo, /32 via LN)
RSC = 16.0      # r1 stream scale: r1 holds 16*LN1 so FFN fp8 weights can be
                # host-scaled into e4m3's normal range (W1 x4, W2 x16); the
                # x16 on both FFN2 residual operands cancels in LN2
W1SC = 4.0

_CACHE = {}


def _build():
    import concourse.mybir as mybir
    import concourse.tile as tile
    from concourse import bacc
    from concourse.masks import make_identity
    from contextlib import ExitStack

    F32 = mybir.dt.float32
    F32R = mybir.dt.float32r
    BF16 = mybir.dt.bfloat16
    F8 = mybir.dt.float8e4
    DR = mybir.MatmulPerfMode.DoubleRow
    AF = mybir.ActivationFunctionType
    OP = mybir.AluOpType

    nc = bacc.Bacc(None, target_bir_lowering=False, debug=False)

    def pairs(ap, n):
        """[P, 2*n] flat pair tile -> [P, 2, n] DoubleRow view."""
        return ap.rearrange("p (two n) -> p two n", two=2)

    with tile.TileContext(nc) as tc:
        es = ExitStack()
        dram = es.enter_context(tc.tile_pool(name="dram", bufs=1, space="DRAM"))

        def din(name, shape, dt=F8):
            return dram.tile(shape, dt, kind="ExternalInput", name=name, uniquify=False)

        x_bf = din("x_bf", [S, D], BF16)      # batch's full sequence (rolled)
        x_own = din("x_own", [T, D], F32)     # 32 * own tokens (residual)
        Wk = din("Wk8", [NC, P, 2 * D]); Wq = din("Wq8", [NC, P, 2 * D])
        Wv = din("Wv8", [NC, P, 2 * D]); Wo = din("Wo8", [NC, P, 2 * D])
        Whv = din("Whv8", [NC, P, 2 * D])
        Whq = din("Whq8", [NHP, P, 1024])
        Whk = din("Whk8", [NHP, P, 1024])
        W1 = din("W18", [8, NC, P, 1024])
        W2 = din("W28", [4 * NC, P, 2 * D])
        bk = din("bk", [D], F32); bq = din("bq", [D], F32); bv = din("bv", [D], F32)
        bhq = din("bhq", [H, HD], F32); bhk = din("bhk", [H, HD], F32)
        bhv = din("bhv", [H, HD], F32R)
        bo = din("bo", [D], F32R); b1 = din("b1", [DFF], F32); b2 = din("b2", [D], F32R)
        out = dram.tile([T, D], F32, kind="ExternalOutput", name="out", uniquify=False)

        # ---------------- constants / psum ----------------
        const = es.enter_context(tc.tile_pool(name="const", bufs=1))
        ident = const.tile([P, P], F32, name="ident")
        make_identity(nc, ident)
        identb = const.tile([P, P], BF16, name="identb")
        nc.vector.tensor_copy(identb[:], ident[:])
        ones_f32 = const.tile([P, 32], F32, name="ones_f32")
        nc.vector.memset(ones_f32[:], 1.0)
        onesf2 = const.tile([P, P], F32, name="onesf2")
        nc.vector.memset(onesf2[:], 1.0)
        ones_r = const.tile([P, P], F32R, name="ones_r")
        nc.scalar.copy(ones_r[:], onesf2[:])
        oinvf = const.tile([1, HD], F32, name="oinvf")
        nc.vector.memset(oinvf[:], 1.0 / OSC)
        oinv_r = const.tile([1, HD], F32R, name="oinv_r")
        nc.scalar.copy(oinv_r[:], oinvf[:])
        eps_t = const.tile([P, 1], F32, name="eps_t")
        nc.vector.memset(eps_t[:], EPS)
        epsr_t = const.tile([P, 1], F32, name="epsr_t")
        nc.vector.memset(epsr_t[:], EPS / (RSC * RSC))

        bo_rt = const.tile([1, D], F32R, name="bo_rt")
        nc.gpsimd.dma_start(out=bo_rt[:], in_=bo[:].rearrange("(o d) -> o d", o=1))
        b2_rt = const.tile([1, D], F32R, name="b2_rt")
        nc.gpsimd.dma_start(out=b2_rt[:], in_=b2[:].rearrange("(o d) -> o d", o=1))
        bhv_rt = const.tile([1, D], F32R, name="bhv_rt")
        nc.gpsimd.dma_start(out=bhv_rt[:], in_=bhv[:].rearrange("(o h) e -> o (h e)", o=1))
        bo_r, b2_r, bhv_r = bo_rt[:], b2_rt[:], bhv_rt[:]

        def bias_cols(name, vec, ncols):
            t = const.tile([P, ncols], F32, name=name)
            nc.gpsimd.dma_start(out=t[:], in_=vec.rearrange("(m p) -> p m", p=P))
            return t

        bk_t = bias_cols("bk_t", bk[:], ND)
        bq_t = bias_cols("bq_t", bq[:], ND)
        bv_t = bias_cols("bv_t", bv[:], ND)
        bhq_t = bias_cols("bhq_t", bhq[:].rearrange("h e -> (h e)"), NHP)
        bhk_t = bias_cols("bhk_t", bhk[:].rearrange("h e -> (h e)"), NHP)
        b1_t = bias_cols("b1_t", b1[:], NF)

        ln_p = es.enter_context(tc.tile_pool(name="ln_p", bufs=3))
        psum = es.enter_context(tc.tile_pool(name="psum", bufs=1, space="PSUM"))

        def ps_tile(name, shape=(P, 512), tag="ps", bufs=4, dt=F32):
            return psum.tile(list(shape), dt, name=name, tag=tag, bufs=bufs)

        ev_i = [0]
        ev_dve_only = [False]

        def evict(dst, src, bias=None):
            """PSUM -> SBUF eviction: 2 of 3 on DVE, 1 of 3 on ACT."""
            i = ev_i[0]; ev_i[0] += 1
            if i % 3 == 2 and not ev_dve_only[0]:
                if bias is None:
                    nc.scalar.copy(dst, src)
                else:
                    nc.scalar.activation(dst, src, AF.Identity, bias=bias)
            else:
                if bias is None:
                    nc.vector.tensor_copy(dst, src)
                else:
                    nc.vector.tensor_scalar_add(dst, src, bias)

        # ------- whole-kernel weight staging pool: 24 rotating 2KB slots ------
        wstage = es.enter_context(tc.tile_pool(name="wstage", bufs=1))

        def wtiles(name, w_dram, n=NC, cols=2 * D):
            sb = []
            for k in range(n):
                wt = wstage.tile([P, cols], F8, name=f"w_{name}{k}", tag="w", bufs=24)
                nc.sync.dma_start(out=wt[:], in_=w_dram[k])
                sb.append(wt)
            return sb

        wk_sb = wtiles("wk", Wk)
        wv_sb = wtiles("wv", Wv)
        whv_sb = wtiles("whv", Whv)
        wq_sb = wtiles("wq", Wq)
        whk_sb = wtiles("whk", Whk, n=NHP, cols=1024)
        whq_sb = wtiles("whq", Whq, n=NHP, cols=1024)

        # right-side persistent pools (bottom: longest-lived)
        posb = ExitStack()
        osb_pool = posb.enter_context(tc.tile_pool(name="osb_pool", bufs=1, side="right"))
        o8 = [osb_pool.tile([P, 2 * T], F8, name=f"o8_{c}") for c in range(NC)]
        pva = ExitStack()
        va_pool = pva.enter_context(tc.tile_pool(name="va_pool", bufs=1, side="right"))
        va8 = [va_pool.tile([P, 2 * H * (HD + 1)], F8, name=f"va8_{c}")
               for c in range(NS // 2)]
        pkt = ExitStack()
        kt_pool = pkt.enter_context(tc.tile_pool(name="kt_pool", bufs=1, side="right"))
        k_t = [kt_pool.tile([P, S], BF16, name=f"kh_o{m}") for m in range(NHP)]
        pqt = ExitStack()
        qt_pool = pqt.enter_context(tc.tile_pool(name="qt_pool", bufs=1, side="right"))
        q_t = [qt_pool.tile([P, T], BF16, name=f"qh_o{m}") for m in range(NHP)]

        # left-side long-lived: ko/qo (read inside the attention loop)
        p_ko = ExitStack()
        ko_pool = p_ko.enter_context(tc.tile_pool(name="ko_pool", bufs=1))
        p_qo = ExitStack()
        qo_pool = p_qo.enter_context(tc.tile_pool(name="qo_pool", bufs=1))

        # ================= Phase A: load x token-major, transpose on PE ========
        pxf = ExitStack()
        xf_p = pxf.enter_context(tc.tile_pool(name="xf_p", bufs=1))
        xf8 = [xf_p.tile([P, 2 * S], F8, name=f"xf8_{c}") for c in range(NC)]
        pxtm = ExitStack()
        xtm_p = pxtm.enter_context(tc.tile_pool(name="xtm_p", bufs=1))
        xtm = [xtm_p.tile([P, D], BF16, name=f"xtm{i}") for i in range(NS)]
        for i in range(NS):
            nc.scalar.dma_start(out=xtm[i][:], in_=x_bf[i * P:(i + 1) * P, :])

        def transpose_x(i_range):
            for i in i_range:
                for j in range(ND):
                    tp = ps_tile(f"tp{i}_{j}", shape=(P, P), tag="ps", dt=BF16)
                    nc.tensor.transpose(tp[:P, :P], xtm[i][:, j * P:(j + 1) * P],
                                        identb[:])
                    evict(xf8[j // 2][:, (j % 2) * S + i * P:
                                      (j % 2) * S + (i + 1) * P], tp[:P, :P])

        transpose_x(range(NT))          # own half first: B0 needs cols [0, T)

        # =============== dense fp8 projection helper ===============
        def wproj8(name, w_sb, n_tok, bias_col, pool_out, src8):
            """[D, D] projection in DoubleRow fp8; pair-tile output."""
            outs = [pool_out.tile([P, 2 * n_tok], F8, name=f"{name}8_{mc}")
                    for mc in range(NC)]
            srcv = [pairs(s[:], S) for s in src8]
            for m in range(ND):
                for n in range(n_tok // 512):
                    ps = ps_tile(f"ps_{name}{m}_{n}")
                    for c in range(NC):
                        nc.tensor.matmul(
                            ps[:],
                            pairs(w_sb[c][:], D)[:, :, m * P:(m + 1) * P],
                            srcv[c][:, :, n * 512:(n + 1) * 512],
                            start=(c == 0), stop=(c == NC - 1), perf_mode=DR)
                    evict(outs[m // 2][:, (m % 2) * n_tok + n * 512:
                                       (m % 2) * n_tok + (n + 1) * 512],
                          ps[:], bias=bias_col[:, m:m + 1])
            return outs

        # =============== Phase B0: Q-stream outer (own tokens = cols [0,T)) ====
        # own-token columns of xf8 are cols [0,T) of each plane; build views
        xo_view = [None] * NC

        class _XoSrc:
            def __init__(self, c):
                self.c = c
            def __getitem__(self, sl):
                return xf8[self.c][sl]

        # ko uses a restricted view: plane i cols [i*S, i*S+T)
        ko8 = [ko_pool.tile([P, 2 * T], F8, name=f"ko8_{mc}") for mc in range(NC)]
        for m in range(ND):
            ps = ps_tile(f"ps_ko{m}")
            for c in range(NC):
                lhsT = pairs(wk_sb[c][:], D)[:, :, m * P:(m + 1) * P]
                rhs = pairs(xf8[c][:], S)[:, :, 0:T]
                nc.tensor.matmul(ps[:], lhsT, rhs, start=(c == 0),
                                 stop=(c == NC - 1), perf_mode=DR)
            evict(ko8[m // 2][:, (m % 2) * T:(m % 2) * T + T], ps[:],
                  bias=bk_t[:, m:m + 1])

        transpose_x(range(NT, NS))      # other half, needed from B1 on
        pxtm.close()

        # =============== Phase B1: V stream -> v_aug ===============
        p_vo = ExitStack()
        vo_pool = p_vo.enter_context(tc.tile_pool(name="vo_pool", bufs=1))
        vo8 = wproj8("vo", wv_sb, S, bv_t, vo_pool, xf8)

        for i in range(NS):
            ic, ip = i // 2, i % 2
            for n in range(2):
                ps = ps_tile(f"vkm{i}_{n}")
                for c in range(NC):
                    nc.tensor.matmul(
                        ps[:],
                        pairs(vo8[c][:], S)[:, :, i * P:(i + 1) * P],
                        pairs(whv_sb[c][:], D)[:, :, n * 512:(n + 1) * 512],
                        start=(c == 0), stop=False, perf_mode=DR)
                nc.tensor.matmul(ps[:], ones_r[:1, 0:P], bhv_r[:, n * 512:(n + 1) * 512],
                                 start=False, stop=True)
                dst = va8[ic][:].rearrange("p (two h e) -> p two h e", two=2, e=HD + 1)
                evict(dst[:, ip:ip + 1, 8 * n:8 * (n + 1), 0:HD],
                      ps[:].rearrange("p (o h e) -> p o h e", o=1, e=HD))
            if ip == 1:
                dst = va8[ic][:].rearrange("p (two h e) -> p two h e", two=2, e=HD + 1)
                nc.vector.tensor_copy(dst[:, :, :, HD:HD + 1],
                                      ones_f32[:, 0:32].rearrange(
                                          "p (two h o) -> p two h o", two=2, o=1))
        p_vo.close()

        # =============== Phase B2: K-stream outer (full sequence) =============
        qo8 = wproj8("qo", wq_sb, S, bq_t, qo_pool, xf8)
        pxf.close()

        # ====== interleaved loop: per head pair, K/Q head proj + attention ======
        pc = ExitStack()
        pkm_p = pc.enter_context(tc.tile_pool(name="pkm", bufs=12))
        den_p = pc.enter_context(tc.tile_pool(name="den_p", bufs=3))
        ev_dve_only[0] = True

        x_tok = [None] * NT
        wo_sb = [None] * NC
        pending = [None, None, None]   # [head, den-row, ops] awaiting normalization

        def finish():
            """Normalize pending head: PE-broadcast the raw denominator (scaled
            1/OSC) over HD rows, full-width DVE reciprocal-evict, multiply."""
            h, den, ops = pending
            hp, hl = h // 2, (h % 2) * HD
            bc = ps_tile(f"bc{h}", shape=(HD, T), tag="bc", bufs=2)
            nc.tensor.matmul(bc[:], oinv_r[:1, :], den[:], start=True, stop=True)
            bcs = den_p.tile([HD, T], F32, name=f"bcs{h}", tag="bcs")
            nc.vector.reciprocal(bcs[:], bc[:])
            nc.vector.tensor_tensor(o8[hp // 2][hl:hl + HD, (hp % 2) * T:
                                                (hp % 2) * T + T],
                                    ops[0:HD, :], bcs[:], op=OP.mult)

        for hp in range(NHP):
            # k_t[hp]: per-head K projection over the full sequence (bf16 out)
            for n in range(2):
                ps = ps_tile(f"ps_kh{hp}_{n}")
                for c in range(NC):
                    nc.tensor.matmul(
                        ps[:],
                        pairs(whk_sb[hp][:, c * 256:(c + 1) * 256], P),
                        pairs(qo8[c][:], S)[:, :, n * 512:(n + 1) * 512],
                        start=(c == 0), stop=(c == NC - 1), perf_mode=DR)
                evict(k_t[hp][:, n * 512:(n + 1) * 512], ps[:],
                      bias=bhk_t[:, hp:hp + 1])
            # q_t[hp]: per-head Q projection over own tokens
            ps = ps_tile(f"ps_qh{hp}")
            for c in range(NC):
                nc.tensor.matmul(
                    ps[:],
                    pairs(whq_sb[hp][:, c * 256:(c + 1) * 256], P),
                    pairs(ko8[c][:], T),
                    start=(c == 0), stop=(c == NC - 1), perf_mode=DR)
            evict(q_t[hp][:], ps[:], bias=bhq_t[:, hp:hp + 1])

            # prefetch Phase D inputs under the attention loop
            if hp == 2:
                for i in range(NT):
                    x_tok[i] = wstage.tile([P, D], F32, name=f"x_tok{i}",
                                           tag="xtok", bufs=NT)
                    nc.gpsimd.dma_start(out=x_tok[i][:],
                                        in_=x_own[i * P:(i + 1) * P, :])
            if hp == 4:
                for c in range(NC):
                    wo_sb[c] = wstage.tile([P, 2 * D], F8, name=f"wo{c}",
                                           tag="w", bufs=24)
                    nc.sync.dma_start(out=wo_sb[c][:], in_=Wo[c])

            # attention for the two heads of this pair
            for h in (2 * hp, 2 * hp + 1):
                hl = (h % 2) * HD
                p_km = []
                for i in range(NS):
                    ps = ps_tile(f"sc{h}_{i}")
                    nc.tensor.matmul(ps[:], k_t[hp][hl:hl + HD, i * P:(i + 1) * P],
                                     q_t[hp][hl:hl + HD, :], start=True, stop=True)
                    if i % 2 == 0:
                        pk = pkm_p.tile([P, 2 * T], F8, name=f"pkm{h}_{i // 2}",
                                        tag="pkm")
                        p_km.append(pk)
                    nc.scalar.activation(p_km[i // 2][:, (i % 2) * T:(i % 2) * T + T],
                                         ps[:], AF.Exp, scale=SCL)
                if pending[0] is not None:
                    finish()
                ops = ps_tile(f"ops{h}", shape=(HD + 1, T), tag="ops", bufs=2)
                for c in range(NS // 2):
                    nc.tensor.matmul(
                        ops[:],
                        pairs(va8[c][:], H * (HD + 1))[:, :, h * (HD + 1):
                                                       (h + 1) * (HD + 1)],
                        pairs(p_km[c][:], T),
                        start=(c == 0), stop=(c == NS // 2 - 1), perf_mode=DR)
                den = den_p.tile([1, T], F32R, name=f"den{h}", tag="den")
                with nc.allow_low_precision(reason="f32r is 4-byte f32 storage"):
                    nc.vector.tensor_copy(den[:], ops[HD:HD + 1, :])
                pending = [h, den, ops]
        finish()
        ev_dve_only[0] = False
        pc.close()
        pqt.close(); pkt.close(); pva.close()
        p_qo.close(); p_ko.close()

        # =============== Phase D: output proj + residual + LN1 ===============
        pr1 = ExitStack()
        r1_pool = pr1.enter_context(tc.tile_pool(name="r1_pool", bufs=1))
        r1 = [r1_pool.tile([P, D], F32, name=f"r1_{i}") for i in range(NT)]
        rt8 = [r1_pool.tile([P, 2 * T], F8, name=f"rt8_{c}") for c in range(NC)]
        pe1 = ExitStack()
        ht_pool = pe1.enter_context(tc.tile_pool(name="ht_pool", bufs=1))
        h8 = [ht_pool.tile([P, 2 * T], F8, name=f"h8_{c}") for c in range(NF // 2)]
        e1s = ExitStack()
        w1_p = e1s.enter_context(tc.tile_pool(name="w1_p", bufs=12))
        w1_first = []
        for c in range(NC):
            wt = w1_p.tile([P, 1024], F8, name=f"w1_0_{c}", tag="w1")
            nc.sync.dma_start(out=wt[:], in_=W1[0, c])
            w1_first.append(wt)
        pd = ExitStack()
        pre_p = pd.enter_context(tc.tile_pool(name="pre_p", bufs=2))

        def layernorm(tag, i, pre, dst, outscale=1.0):
            """dst = outscale * LN(pre) along free dim (D=1024).

            outscale folds into the rsqrt: sd' = sqrt(var + eps)/outscale via
            the Sqrt activation's input scale, so the scaled LN costs nothing.
            """
            st = ln_p.tile([P, 12], F32, name=f"st{tag}{i}", tag="st")
            nc.vector.bn_stats(st[:, 0:6], pre[:, 0:512])
            nc.vector.bn_stats(st[:, 6:12], pre[:, 512:1024])
            ag = ln_p.tile([P, 2], F32, name=f"ag{tag}{i}", tag="ag")
            nc.vector.bn_aggr(ag[:], st[:].rearrange("p (n s) -> p n s", n=2))
            sd = ln_p.tile([P, 1], F32, name=f"sd{tag}{i}", tag="sd")
            if outscale == 1.0:
                nc.scalar.activation(sd[:], ag[:, 1:2], AF.Sqrt, bias=eps_t[:])
            else:
                nc.scalar.activation(sd[:], ag[:, 1:2], AF.Sqrt, bias=epsr_t[:],
                                     scale=1.0 / (outscale * outscale))
            rs = ln_p.tile([P, 1], F32, name=f"rs{tag}{i}", tag="rs")
            nc.vector.reciprocal(rs[:], sd[:])
            nc.vector.tensor_scalar(dst, pre[:], ag[:, 0:1], rs[:],
                                    op0=OP.subtract, op1=OP.mult)

        def d_proj(i):
            pre = pre_p.tile([P, D], F32, name=f"pre1_{i}", tag="pre1")
            for n in range(2):
                ps = ps_tile(f"at{i}_{n}")
                for c in range(NC):
                    nc.tensor.matmul(
                        ps[:],
                        pairs(o8[c][:], T)[:, :, i * P:(i + 1) * P],
                        pairs(wo_sb[c][:], D)[:, :, n * 512:(n + 1) * 512],
                        start=(c == 0), stop=False, perf_mode=DR)
                nc.tensor.matmul(ps[:], ones_r[:1, 0:P], bo_r[:, n * 512:(n + 1) * 512],
                                 start=False, stop=True)
                nc.vector.tensor_tensor(pre[:, n * 512:(n + 1) * 512], ps[:],
                                        x_tok[i][:, n * 512:(n + 1) * 512], op=OP.add)
            layernorm("r", i, pre, r1[i][:], outscale=RSC)

        def d_transpose(i):
            for j in range(ND):
                tp = ps_tile(f"r1tp{j}_{i}", shape=(P, P), tag="ops", bufs=2)
                nc.tensor.transpose(tp[:P, :P], r1[i][:, j * P:(j + 1) * P], ident[:])
                nc.scalar.copy(rt8[j // 2][:, (j % 2) * T + i * P:
                                           (j % 2) * T + (i + 1) * P], tp[:P, :P])

        d_proj(0)
        d_proj(1)
        d_transpose(0)
        d_proj(2)
        d_transpose(1)
        d_proj(3)
        d_transpose(2)
        d_transpose(3)
        pd.close()
        posb.close()

        # =============== Phase E: FFN1 (stream W1, prefetch W2) ===============
        w2_sb = [None] * (4 * NC)
        for blk in range(8):            # dff blocks of 512
            if blk == 0:
                w1_sb = w1_first
            else:
                w1_sb = []
                for c in range(NC):
                    wt = w1_p.tile([P, 1024], F8, name=f"w1_{blk}_{c}", tag="w1")
                    nc.sync.dma_start(out=wt[:], in_=W1[blk, c])
                    w1_sb.append(wt)
            # interleave W2 prefetch (2 tiles per block) on the same queue
            for c in range(2 * blk, 2 * blk + 2):
                w2_sb[c] = wstage.tile([P, 2 * D], F8, name=f"w2_{c}", tag="w",
                                       bufs=24)
                nc.sync.dma_start(out=w2_sb[c][:], in_=W2[c])
            for mm in range(4):         # 128-chunks within the block
                m = blk * 4 + mm
                ps = ps_tile(f"ff1_{m}")
                for c in range(NC):
                    nc.tensor.matmul(
                        ps[:],
                        pairs(w1_sb[c][:], 512)[:, :, mm * P:(mm + 1) * P],
                        pairs(rt8[c][:], T),
                        start=(c == 0), stop=(c == NC - 1), perf_mode=DR)
                # psum = (16 r1) @ (4 W1) = 64 * (r1 @ W1); Gelu's input scale
                # restores the true pre-activation exactly
                nc.scalar.activation(h8[m // 2][:, (m % 2) * T:(m % 2) * T + T],
                                     ps[:], AF.Gelu, bias=b1_t[:, m:m + 1],
                                     scale=1.0 / (RSC * W1SC))
        e1s.close()

        # =============== Phase E2: FFN2 per output tile (W2 resident) =========
        pout = ExitStack()
        out_p = pout.enter_context(tc.tile_pool(name="out_p", bufs=2))
        tags = [("ps", 4), ("ps", 4), ("ops", 2), ("bc", 2)]
        for i in range(NT):
            tag, bufs = tags[i]
            pss = [ps_tile(f"ff2_{i}_{n}", shape=(P, 512), tag=tag, bufs=bufs)
                   for n in range(2)]
            for c in range(4 * NC):
                for n in range(2):
                    nc.tensor.matmul(
                        pss[n][:],
                        pairs(h8[c][:], T)[:, :, i * P:(i + 1) * P],
                        pairs(w2_sb[c][:], D)[:, :, n * 512:(n + 1) * 512],
                        start=(c == 0), stop=False, perf_mode=DR)
            pre = out_p.tile([P, D], F32, name=f"pre2_{i}", tag="pre2")
            for n in range(2):
                nc.tensor.matmul(pss[n][:], ones_r[:1, 0:P],
                                 b2_r[:, n * 512:(n + 1) * 512], start=False, stop=True)
                nc.vector.tensor_tensor(pre[:, n * 512:(n + 1) * 512], pss[n][:],
                                        r1[i][:, n * 512:(n + 1) * 512],
                                        op=OP.add)
            o_sb2 = out_p.tile([P, D], F32, name=f"osb2_{i}", tag="osb2")
            layernorm("o", i, pre, o_sb2[:])
            nc.gpsimd.dma_start(out=out[i * P:(i + 1) * P, :], in_=o_sb2[:])
        pout.close()
        pe1.close()
        pr1.close()

        es.close()
    nc.compile()
    return nc


def _get_program():
    if "nc" not in _CACHE:
        _CACHE["nc"] = _build()
    return _CACHE["nc"]


def _prepack(inputs):
    """Quantize weights to TRN e4m3 and prepack into DoubleRow pair layouts."""
    import ml_dtypes
    f8 = ml_dtypes.float8_e4m3

    def q8(a):
        a = np.asarray(a, dtype=np.float32)
        return np.ascontiguousarray(np.clip(a, -240.0, 240.0).astype(f8))

    def pair(W):
        """[D, N] -> [NC, P, 2N]: out[c, p, i*N+n] = W[256c+128i+p, n]."""
        N = W.shape[1]
        return W.reshape(NC, 2, P, N).transpose(0, 2, 1, 3).reshape(NC, P, 2 * N)

    Wk = np.asarray(inputs["Wk"], np.float32)
    Wq = np.asarray(inputs["Wq"], np.float32)
    Wv = np.asarray(inputs["Wv"], np.float32)
    Wo = np.asarray(inputs["Wo"], np.float32)
    Whq = np.asarray(inputs["Whq"], np.float32)
    Whk = np.asarray(inputs["Whk"], np.float32)
    Whv = np.asarray(inputs["Whv"], np.float32)
    W1 = np.asarray(inputs["W1"], np.float32)
    W2 = np.asarray(inputs["W2"], np.float32)

    # Whv feature-major: [d, h*64+e]
    whv_fm = Whv.transpose(1, 0, 2).reshape(D, D)
    # Whk/Whq: [hp][p, c2*256 + i*128 + h'*64 + e] = Wh[2hp+h', 256c2+128i+p, e]
    def head_pair(Wh):
        a = Wh.reshape(NHP, 2, NC, 2, P, HD)        # [hp, h', c2, i, p, e]
        return a.transpose(0, 4, 2, 3, 1, 5).reshape(NHP, P, 1024)
    # W1: [blk, c, p, i*512+j] = W1[256c+128i+p, 512blk+j]
    w18 = W1.reshape(NC, 2, P, 8, 512).transpose(3, 0, 2, 1, 4).reshape(8, NC, P, 1024)
    # W2: [c(16), p, i*D+fo] = W2[256c+128i+p, fo]
    w28 = W2.reshape(4 * NC, 2, P, D).transpose(0, 2, 1, 3).reshape(4 * NC, P, 2 * D)

    f32 = lambda n: np.ascontiguousarray(inputs[n], dtype=np.float32)
    return {
        "Wk8": q8(pair(Wk)), "Wq8": q8(pair(Wq)), "Wv8": q8(pair(Wv)),
        "Wo8": q8(pair(Wo * 2.0)),      # x2: keeps Wo normal-range in e4m3
        "Whv8": q8(pair(whv_fm)),
        "Whq8": q8(head_pair(Whq)), "Whk8": q8(head_pair(Whk)),
        "W18": q8(w18 * W1SC), "W28": q8(w28 * RSC),
        "bk": f32("bk"), "bq": f32("bq"), "bv": f32("bv"),
        "bhq": f32("bhq"), "bhk": f32("bhk"), "bhv": f32("bhv"),
        "bo": f32("bo") * 32.0,         # matches 16*o x 2*Wo scaling
        "b1": f32("b1"),
        "b2": f32("b2") * RSC,          # FFN2 stream carries x16
    }


def _in_maps(inputs):
    import ml_dtypes
    x = np.ascontiguousarray(inputs["x"], dtype=np.float32)
    x_bf = x.astype(ml_dtypes.bfloat16)
    wmap = _prepack(inputs)
    in_maps = []
    for c in range(8):
        b_, half = c // 2, c % 2
        m = dict(wmap)
        m["x_bf"] = np.ascontiguousarray(np.roll(x_bf[b_], -half * T, axis=0))
        m["x_own"] = x[b_, half * T:(half + 1) * T] * 32.0
        in_maps.append(m)
    return in_maps


def kernel(**inputs):
    from concourse.bass_utils import run_bass_kernel_spmd

    nc = _get_program()
    res = run_bass_kernel_spmd(nc, _in_maps(inputs), core_ids=list(range(8)))
    y = np.empty((B, S, D), dtype=np.float32)
    for c in range(8):
        b_, half = c // 2, c % 2
        y[b_, half * T:(half + 1) * T] = res.results[c]["out"]
    return y
